# revision 53
# baseline (speedup 1.0000x reference)
"""Trainium2 Bass kernel for a binarized (1w1a) ResNet BasicBlock.

Computation (eval mode):
    out = hardtanh(bn2(conv2(sign(out1)) * alpha2) + x)
    out1 = hardtanh(bn1(conv1(sign(x)) * alpha1))
with conv_k a 3x3 stride-1 pad-1 conv whose weights are binarized to
sign(w - rowmean(w)).  Binary operands are exact in fp8e4m3 and PSUM
accumulation is fp32, so the conv arithmetic is exact.

Layout / strategy:
 - Data-parallel over batch N=64 -> 8 images per NeuronCore.
 - Channels 256 = 2 chunks of 128 partitions.  DoubleRow fp8 matmuls
   contract over both chunks at once (K=256): lhsT [128, 2, 128],
   rhs [128, 2, rows, cols].
 - Each conv = 9 shifted matmuls (3x3 taps) accumulated into PSUM per
   (co_chunk, 16-row half).  Boundary taps use reduced row AND column
   ranges (no zero-padding needed at all); the first tap (dh=dw=0)
   covers the full half so start=True clears every has_written bit.
 - Weight prep per (conv, co_chunk), no PE/PSUM involvement: DMA ->
   tap reduce (DVE) -> partition all-reduce (GPSIMD) -> compare vs
   mean.  conv1 weights become {-.5,+.5} via fused is_ge ops on
   DVE/Pool (s1 doubled); conv2 chunk-0 weights {-1,+1} via subtract +
   ACT Sign, chunk-1 {-.5,+.5} via DVE/Pool is_ge (s2 of that chunk
   doubled) — chunk 1's encode would otherwise queue behind the x
   binarizations + ba2 posts on the saturated ACT and stall back(0,1).
   The conv matmul lhsT reads sgn[:, :, :, t] directly (strided AP,
   no repack copies).
 - conv1 -> bn -> hardtanh -> sign fuses to one ACT op per half:
   ba2 = sign(s1*psum + t1).
 - conv2 epilogue: xp = x + t2 per image-chunk on GPSIMD (readiness-
   gated behind conv1(n) so Pool stays clear in the DMA-bound
   transition window); per half one DVE scalar_tensor_tensor
   v = s2*psum + xp (bf16 out) and one clamp; store bf16 (host
   converts to fp32; 2^-9 rounding << 2e-2 tol).
 - The PE executes in emission order, other engines dispatch by
   readiness (with a 4-deep wait-queue bypass): conv work is emitted at
   chunk granularity in an explicit token program matching dependency
   readiness (x DMA + sign for conv1, weight-prep chains for conv2);
   all DMA on SP HWDGE in first-need order: w1, bn, x0, x1, w2c0, x2,
   w2c1, x3, x4...  The x0/x1 chunk-1 transfers are split in halves so
   their ACT signs (the head's last gate) start one transfer earlier.
 - PE warmup: 16 garbage matmuls at t~1us plus tiny "warm gate" pings
   (one per w1-quarter landing, one on sgn1_c0, one on x0c0) keep the
   PE busy-run alive through the DMA-bound head, so the p-state ramp
   (low->mid->full over ~4.8us of PE busy-run age) is fully burned
   before the first real conv matmul; all conv matmuls run at full
   clock.  The garbage inputs are memset-zero fp8 tiles; the scratch
   PSUM slot is start=True-cleared by later conv groups.
 - Tail: the last two images' conv2 chunk 1 runs with 8-row blocks
   whose epilogues pipeline with the conv; each 16-row half is
   assembled in a persistent bf16 tile and stored with a single DMA
   (more, smaller stores would serialize on the 625ns HWDGE
   descriptor device).
 - Steady state: conv2 (back) chunks lead the conv1 chunks by one
   slot so ready conv2 work covers every ba1(n)-sign and w2-encode
   wait; c1(3,1) covers the w2c1-encode latency for back(0,1).
"""

import numpy as np

import concourse.bass as bass  # noqa: F401  (AP types referenced via APIs)
import concourse.bass_isa as bass_isa
import concourse.mybir as mybir
import concourse.tile as tile
from concourse import bacc
from concourse.bass_utils import run_bass_kernel_spmd

N_CORES = 8
IMGS = 8  # images per core
CH = 2  # channel chunks of 128
P = 128
H = 32
W = 32
PIX = H * W
BASTRIDE = PIX + 16  # slack so boundary-tap AP slicing stays in bounds
HALF = 16  # rows per output half (psum free = 16*32 = 512)
EPS = 1e-5
FP = mybir.dt.float32
BF = mybir.dt.bfloat16
F8 = mybir.dt.float8e4
AF = mybir.ActivationFunctionType
DR = mybir.MatmulPerfMode.DoubleRow

# dh=dw=0 first: the first matmul of each accumulation group must cover
# the full half (start=True clears the whole bank's has_written bits)
TAPS = [4, 3, 5, 1, 7, 0, 2, 6, 8]

# superseded by the xpadd gate (kept for experiments): images whose xp
# emission is deferred to back(n-2)
XP_DEFER = ()


def _tap_geom(hs, t, nr):
    """Valid local output rows [r0,r1) and cols [c0,c1) for tap t."""
    dh, dw = t // 3 - 1, t % 3 - 1
    r0 = max(0, -(hs + dh))
    r1 = min(nr, H - hs - dh)
    c0 = max(0, -dw)
    c1 = min(W, W - dw)
    return dh, dw, r0, r1, c0, c1


def _conv_chunk(nc, pspool, wdr, conv, ba, c, consumer, rblock=HALF):
    """One co-chunk of a binarized 3x3 conv over one image (DoubleRow fp8).

    ba: [P, 2, BASTRIDE] fp8 tile; chunk i at [:, i, 0:PIX], rows at
    stride W (no padding columns - boundary taps trim rows/cols).
    consumer(c, h, hs, nr, ps) reads the [P, nr*W] fp32 PSUM tile.
    rblock < HALF splits the accumulation into smaller row blocks so the
    epilogue pipelines with the conv (used to shorten the final tail).
    """
    for h in range(H // rblock):
        hs = h * rblock
        ps = pspool.tile([P, rblock * W], FP, tag="ps", name=f"ps{conv}_{c}_{h}")
        psv = ps.rearrange("p (r w) -> p r w", r=rblock)
        for it, t in enumerate(TAPS):
            dh, dw, r0, r1, c0, c1 = _tap_geom(hs, t, rblock)
            s = (hs + r0 + dh) * W + c0 + dw
            rhs = ba[:, :, s : s + (r1 - r0) * W].rearrange(
                "p i (r w) -> p i r w", r=r1 - r0
            )[:, :, :, 0 : c1 - c0]
            nc.tensor.matmul(
                psv[:, r0:r1, c0:c1],
                wdr[(conv, t, c)],
                rhs,
                start=(it == 0),
                stop=(it == 8),
                perf_mode=DR,
                skip_group_check=True,
            )
        consumer(c, h, hs, rblock, ps)


def build_program(loop_r=None, pipelined=True, loop_scope="images",
                  half_imgs=frozenset(), tail_rb=8, prog_variant=0,
                  w2_enc="ts1", tail_rb0=False, split_x1=True,
                  reorder_x2=False, half_mode="half"):
    """loop_r: if set, wrap the workload in For_i(0, loop_r) -
    benchmarking only (re-processes the same images each iteration).
    loop_scope: 'images' loops only the image pipeline; 'all' also loops
    weight prep + BN constant computation (approximates single-shot cost)."""
    nc = bacc.Bacc("TRN2", target_bir_lowering=False, debug=False, num_devices=N_CORES)

    x_ext = nc.dram_tensor("x", [IMGS, CH, P, PIX], FP, kind="ExternalInput").ap()
    w_ext = {}
    bn_ext = {}
    for i in (1, 2):
        # transposed layout from host: [ci_chunk, ci%128, co, tap]
        w_ext[i] = nc.dram_tensor(
            f"conv{i}_w", [CH, P, CH * P, 9], FP, kind="ExternalInput"
        ).ap()
        # packed BN+alpha: [co%128, co_chunk*5 + {alpha,gamma,beta,mean,var}]
        bn_ext[i] = nc.dram_tensor(
            f"bn{i}_all", [P, CH * 5], FP, kind="ExternalInput"
        ).ap()
    out_ext = nc.dram_tensor("out", [IMGS, CH, P, PIX], BF, kind="ExternalOutput").ap()

    with tile.TileContext(nc) as tc:
        from contextlib import ExitStack

        with ExitStack() as ctx:
            singles = ctx.enter_context(tc.tile_pool(name="singles", bufs=1))
            wpool = ctx.enter_context(tc.tile_pool(name="wpool", bufs=1))
            wstage = ctx.enter_context(tc.tile_pool(name="wstage", bufs=2))
            xpool = ctx.enter_context(tc.tile_pool(name="xpool", bufs=5))
            xppool = ctx.enter_context(tc.tile_pool(name="xppool", bufs=5))
            bapool = ctx.enter_context(tc.tile_pool(name="bapool", bufs=5))
            vpool = ctx.enter_context(tc.tile_pool(name="vpool", bufs=6))
            pspool = ctx.enter_context(tc.tile_pool(name="psum", bufs=8, space="PSUM"))

            eps_t = singles.tile([P, 1], FP)
            nc.vector.memset(eps_t, EPS)

            def warmup(n_mm=16, width=512):
                """Emit garbage matmuls at t~1us so the sim's PE p-state
                ramp (low->mid->full over ~4.8us from the start of the
                current PE busy run) is burned while the PE would be idle
                waiting on weight DMA+prep.  Together with warm_gate()
                pings this keeps the PE "run" alive until the first real
                conv matmul, which then runs at full speed.  Inputs are a
                memset fp8 tile (products are 0); the PSUM scratch uses
                the shared "ps" ring and is never read."""
                wz = singles.tile([P, CH, width], F8, tag="wuw", name="wuw")
                nc.gpsimd.memset(wz, 0.0)
                ps = pspool.tile([P, width], FP, tag="ps", name="wups")
                for _ in range(n_mm):
                    nc.tensor.matmul(
                        ps, wz[:, :, 0:P], wz, start=True, stop=True,
                        perf_mode=DR, skip_group_check=True,
                    )

            def warm_gate(gate, fp32=True):
                """One tiny matmul whose rhs/lhsT read `gate` (a [128, >=128]
                fp32 AP, or fp8 with fp32=False): becomes READY when the
                gate tile is written, pinging the PE so its busy-run (and
                thus the p-state ramp credit) survives the head's idle
                stretches between DMA arrivals."""
                ps = pspool.tile([P, 16], FP, tag="ps", name="wgps")
                nc.tensor.matmul(
                    ps, gate[:, 0:P], gate[:, 0:16], start=True, stop=True,
                    skip_group_check=True,
                )

            # ---- weight prep (transposed host layout [ci, co, tap]):
            # binarize vs the per-co mean over (ci, tap).  Pipelined per
            # co-chunk so conv1's first matmuls can start before the whole
            # weight tensor is processed; see prep_math for the engine
            # split.  The conv matmul lhsT reads sgn[:, :, :, t] directly
            # (strided AP).
            wdr = {}
            s_t = {}
            t_t = {}

            wstg = {}

            def prep_dma_q(i, c, b):
                """One quarter (ci-chunk b of co-chunk c) of conv i's raw
                weights.  Quarter-granularity lets x transfers interleave
                into the weight stream on the serial DMA device."""
                if (i, c) not in wstg:
                    wstg[(i, c)] = wstage.tile(
                        [P, CH, P, 9], FP, tag=f"wtraw{c}", name=f"wtraw{i}_{c}"
                    )
                nc.sync.dma_start(
                    out=wstg[(i, c)][:, b], in_=w_ext[i][b][:, c * P : (c + 1) * P]
                )

            def prep_dma(i, c):
                for b in range(CH):
                    prep_dma_q(i, c, b)

            def prep_reduce(i, c):
                # tap-reduce (DVE) of the raw weight tile; emitted for both
                # co-chunks before any encode so the chunk-1 reduces are not
                # queued behind chunk-0's DVE encode ops.
                wT = wstg[(i, c)]
                tap = wstage.tile([P, CH, P], FP, tag=f"tap{c}")
                for b in range(CH):
                    nc.vector.tensor_reduce(
                        out=tap[:, b], in_=wT[:, b], axis=mybir.AxisListType.X,
                        op=mybir.AluOpType.add,
                    )
                wstg[(i, c, "tap")] = tap

            def prep_math(i, c):
                # mean via gpsimd partition all-reduce of the tap sums:
                # no PE matmuls, no PSUM — the PE queue stays pure conv work.
                # Compare w*2304 >= colsum instead of w >= mean (same ulp-level
                # rounding class as an explicit division).
                if (i, c, "tap") not in wstg:
                    prep_reduce(i, c)
                wT = wstg.pop((i, c))
                tap = wstg.pop((i, c, "tap"))
                asum = wstage.tile([P, CH, P], FP, tag=f"asum{c}")
                nc.gpsimd.partition_all_reduce(
                    asum, tap, channels=P, reduce_op=bass_isa.ReduceOp.add
                )
                csum = wstage.tile([P, P], FP, tag=f"csum{c}")
                nc.gpsimd.tensor_tensor(
                    out=csum, in0=asum[:, 0], in1=asum[:, 1], op=mybir.AluOpType.add
                )
                sgn = wpool.tile(
                    [P, CH, P, 9], F8, tag=f"sgn{i}_{c}", name=f"sgn{i}_{c}"
                )
                mean = wstage.tile([P, P], FP, tag=f"mean{c}")
                nc.gpsimd.tensor_scalar_mul(mean, csum, 1.0 / (CH * P * 9))
                if i == 1:
                    # conv1 (head-critical): bw in {-.5,+.5} (s1 is doubled),
                    # avoiding ACT which is busy with the x binarizations.
                    # b0 on DVE: fused (w*2304 >= colsum) then -0.5.
                    nc.vector.scalar_tensor_tensor(
                        out=sgn[:, 0], in0=wT[:, 0], scalar=float(CH * P * 9),
                        in1=csum.to_broadcast([P, P, 9]),
                        op0=mybir.AluOpType.mult, op1=mybir.AluOpType.is_ge,
                    )
                    nc.vector.tensor_scalar(
                        out=sgn[:, 0], in0=sgn[:, 0], scalar1=0.5, scalar2=None,
                        op0=mybir.AluOpType.subtract,
                    )
                    # b1: d = w - mean (sign-exact), fused (d >= 0) - 0.5;
                    # split by co-half across Pool and DVE — the Pool
                    # tensor_tensor runs at 0.42 efficiency and this chain
                    # is the head-critical path
                    d = wstage.tile([P, P, 9], FP, tag=f"d1_{c}")
                    hb = P // 2
                    for q, eng in ((0, nc.gpsimd), (1, nc.vector)):
                        sl = slice(q * hb, (q + 1) * hb)
                        eng.tensor_tensor(
                            out=d[:, sl], in0=wT[:, 1, sl],
                            in1=mean[:, sl].to_broadcast([P, hb, 9]),
                            op=mybir.AluOpType.subtract,
                        )
                        eng.tensor_scalar(
                            out=sgn[:, 1, sl], in0=d[:, sl], scalar1=0.0,
                            scalar2=0.5, op0=mybir.AluOpType.is_ge,
                            op1=mybir.AluOpType.subtract,
                        )
                else:
                    # conv2: d on DVE/Pool, then either sign to +-1 on ACT
                    # (w2_enc="act") or the {-.5,+.5} fused is_ge encode on
                    # DVE/Pool (w2_enc="ts", s2 doubled) to keep ACT free
                    # for the x binarizations + ba2 posts in the congested
                    # transition window.  b1's subtract is co-half split
                    # across Pool+DVE (Pool tensor_tensor runs at 0.42
                    # efficiency; this chain gates back(0))
                    hb = P // 2
                    for b in range(CH):
                        d = wstage.tile([P, P, 9], FP, tag=f"d{b}_{c}")
                        if b == 0:
                            nc.vector.tensor_tensor(
                                out=d, in0=wT[:, b],
                                in1=mean.to_broadcast([P, P, 9]),
                                op=mybir.AluOpType.subtract,
                            )
                        else:
                            for q, eng in ((0, nc.gpsimd), (1, nc.vector)):
                                sl = slice(q * hb, (q + 1) * hb)
                                eng.tensor_tensor(
                                    out=d[:, sl], in0=wT[:, b, sl],
                                    in1=mean[:, sl].to_broadcast([P, hb, 9]),
                                    op=mybir.AluOpType.subtract,
                                )
                        if w2_enc == "act" or (w2_enc == "ts1" and c == 0):
                            nc.scalar.sign(sgn[:, b], d)
                        else:
                            for q, eng in ((0, nc.gpsimd), (1, nc.vector)):
                                sl = slice(q * hb, (q + 1) * hb)
                                eng.tensor_scalar(
                                    out=sgn[:, b, sl], in0=d[:, sl],
                                    scalar1=0.0, scalar2=0.5,
                                    op0=mybir.AluOpType.is_ge,
                                    op1=mybir.AluOpType.subtract,
                                )
                for t in range(9):
                    wdr[(i, t, c)] = sgn[:, :, :, t]

            # ---- BN constants: s = alpha*gamma/sqrt(var+eps),
            #                    t = beta - mean*gamma/sqrt(var+eps)
            def prep_bn(i):
                bn_t = singles.tile([P, CH * 5], FP, tag=f"bn{i}", name=f"bn{i}")
                nc.sync.dma_start(out=bn_t, in_=bn_ext[i])
                for c in range(CH):
                    ld = {
                        nm: bn_t[:, c * 5 + k : c * 5 + k + 1]
                        for k, nm in enumerate(("alpha", "gamma", "beta", "mean", "var"))
                    }
                    std = singles.tile([P, 1], FP, tag=f"std{i}_{c}", name=f"std{i}_{c}")
                    nc.scalar.activation(std, ld["var"], AF.Sqrt, bias=eps_t)
                    g = singles.tile([P, 1], FP, tag=f"g{i}_{c}", name=f"g{i}_{c}")
                    nc.vector.reciprocal(g, std)
                    nc.vector.tensor_mul(g, g, ld["gamma"])
                    s = singles.tile([P, 1], FP, tag=f"s{i}_{c}", name=f"s{i}_{c}")
                    nc.vector.tensor_mul(s, g, ld["alpha"])
                    if i == 1:
                        nc.vector.tensor_add(s, s, s)  # x2: conv1 bw +-0.5
                        s4 = singles.tile(
                            [P, 1], FP, tag=f"s4_{c}", name=f"s4_{c}"
                        )
                        # x4 scale for images whose ba1 is half-encoded
                        nc.vector.tensor_add(s4, s, s)
                        s_t[("1x2", c)] = s4
                    elif w2_enc == "ts" or (w2_enc == "ts1" and c == 1):
                        nc.vector.tensor_add(s, s, s)  # x2: conv2 bw +-0.5
                    tt = singles.tile([P, 1], FP, tag=f"t{i}_{c}", name=f"t{i}_{c}")
                    nc.vector.tensor_mul(tt, g, ld["mean"])
                    nc.vector.tensor_sub(tt, ld["beta"], tt)
                    s_t[(i, c)] = s
                    t_t[(i, c)] = tt

            # ---- per-image stages -------------------------------------
            def xload(n):
                """x(n) DMA triggers on the SP queue.  Image 0's second
                chunk is split in two transfers so its binarization (the
                head's last gate) can start on the first half sooner."""
                xt = {}
                for b in range(CH):
                    xt[b] = xpool.tile([P, PIX], FP, tag=f"x{b}", name=f"x{b}")
                    nc.sync.dma_start(out=xt[b], in_=x_ext[n, b])
                return xt

            def basign(state, mode="act"):
                """ba1(n): +-1 sign on ACT ("act"), or the {-.5,+.5}
                encoding via one fused tensor_scalar per chunk — "half"
                puts both chunks on DVE (plain-ts runs at the 2x_2p rate
                there), "half-mixed" splits DVE/Pool.  conv1's post
                compensates half encodings with a doubled scale."""
                xt = state
                half = mode != "act"
                ba1 = bapool.tile([P, CH, BASTRIDE], F8, tag="ba1", name="ba1")
                for b in range(CH):
                    if half:
                        eng = (
                            nc.gpsimd
                            if (mode == "half-mixed" and b == 1)
                            else nc.vector
                        )
                        eng.tensor_scalar(
                            out=ba1[:, b, 0:PIX], in0=xt[b], scalar1=0.0,
                            scalar2=0.5, op0=mybir.AluOpType.is_ge,
                            op1=mybir.AluOpType.subtract,
                        )
                    elif xt.get(("split", b)):
                        hp = PIX // 2
                        for q in range(2):
                            nc.scalar.sign(
                                ba1[:, b, q * hp : (q + 1) * hp],
                                xt[b][:, q * hp : (q + 1) * hp],
                            )
                    else:
                        nc.scalar.sign(ba1[:, b, 0:PIX], xt[b])
                return xt, ba1, half

            def xpadd(xt, gate=None):
                """xp(n) = x(n) + t2 on GPSIMD (residual + BN shift).

                gate: an AP written late (e.g. a ba2 column).  The t2 scalar
                is routed through a tiny gate op that reads it, so the xp
                ops only become READY once conv1(n) is under way — keeping
                Pool free during the DMA-bound transition window (the
                scheduler dispatches by readiness, not emission order).
                """
                xp = {}
                for b in range(CH):
                    t2b = t_t[(2, b)]
                    if gate is not None:
                        t2l = xppool.tile([P, 1], FP, tag=f"t2l{b}", name=f"t2l{b}")
                        nc.vector.scalar_tensor_tensor(
                            out=t2l, in0=gate, scalar=0.0, in1=t2b,
                            op0=mybir.AluOpType.mult, op1=mybir.AluOpType.add,
                        )
                        t2b = t2l
                    xp[b] = xppool.tile([P, PIX], FP, tag=f"xp{b}", name=f"xp{b}")
                    nc.gpsimd.tensor_scalar_add(xp[b], xt[b], t2b)
                return xp

            def make_c1(n, state):
                """conv1(n) -> ba2(n); returns (shared-state, chunk-emitter)."""
                xt, ba1, half = state
                skey = "1x2" if half else 1
                st = {}

                def chunk(c):
                    first = not st
                    if first:
                        st["ba2"] = bapool.tile(
                            [P, CH, BASTRIDE], F8, tag="ba2", name="ba2"
                        )
                        st["xt"] = xt

                    def conv1_post(cc, h, hs, nr, ps):
                        # ba2 = sign(s1*conv + t1)  (sign(hardtanh(y))==sign(y))
                        nc.scalar.activation(
                            st["ba2"][:, cc, hs * W : hs * W + nr * W],
                            ps,
                            AF.Sign,
                            bias=t_t[(1, cc)],
                            scale=s_t[(skey, cc)],
                        )

                    _conv_chunk(nc, pspool, wdr, 1, ba1, c, conv1_post)
                    if c == 1:
                        # emitted after the second chunk so the gate read
                        # depends on the last ba2 write (see xpadd)
                        st["xp"] = xpadd(
                            xt, gate=st["ba2"][:, 1, PIX - 1 : PIX]
                        )

                return st, chunk

            def make_back(n, st, last=False):
                def chunk(c):
                    if "xp" not in st:
                        st["xp"] = xpadd(st["xt"])
                    def conv2_post(cc, h, hs, nr, ps):
                        # last image's epilogue is the tail: put the clamp of
                        # alternating halves on Pool (GPSIMD cannot read
                        # PSUM, so the stt stays on DVE)
                        eng = nc.gpsimd if (last and (cc + h) % 2) else nc.vector
                        v = vpool.tile([P, nr * W], BF, tag="v", name="v")
                        nc.vector.scalar_tensor_tensor(
                            out=v, in0=ps, scalar=s_t[(2, cc)],
                            in1=st["xp"][cc][:, hs * W : hs * W + nr * W],
                            op0=mybir.AluOpType.mult, op1=mybir.AluOpType.add,
                        )
                        eng.tensor_scalar(
                            out=v, in0=v, scalar1=1.0, scalar2=-1.0,
                            op0=mybir.AluOpType.min, op1=mybir.AluOpType.max,
                        )
                        nc.sync.dma_start(
                            out=out_ext[n, cc][:, hs * W : hs * W + nr * W],
                            in_=v,
                        )

                    if last and (c == 1 or tail_rb0) and tail_rb:
                        # pipeline the final chunk's epilogue with its conv
                        # (4-row blocks) but keep one store per 16-row half:
                        # 8 small stores would serialize on the 625ns HWDGE
                        vh = {}

                        def conv2_post_last(cc, h, hs, nr, ps):
                            q = hs // HALF  # which 16-row half
                            if q not in vh:
                                vh[q] = vpool.tile(
                                    [P, HALF * W], BF, tag="v", name="vlast"
                                )
                            ro = (hs % HALF) * W
                            sl = slice(ro, ro + nr * W)
                            v = vh[q]
                            nc.vector.scalar_tensor_tensor(
                                out=v[:, sl], in0=ps, scalar=s_t[(2, cc)],
                                in1=st["xp"][cc][:, hs * W : hs * W + nr * W],
                                op0=mybir.AluOpType.mult,
                                op1=mybir.AluOpType.add,
                            )
                            eng = nc.gpsimd if h % 2 else nc.vector
                            eng.tensor_scalar(
                                out=v[:, sl], in0=v[:, sl], scalar1=1.0,
                                scalar2=-1.0, op0=mybir.AluOpType.min,
                                op1=mybir.AluOpType.max,
                            )
                            if hs + nr == (q + 1) * HALF:
                                nc.sync.dma_start(
                                    out=out_ext[n, cc][
                                        :, q * HALF * W : (q + 1) * HALF * W
                                    ],
                                    in_=v,
                                )

                        _conv_chunk(nc, pspool, wdr, 2, st["ba2"], c,
                                    conv2_post_last, rblock=tail_rb)
                    else:
                        _conv_chunk(nc, pspool, wdr, 2, st["ba2"], c,
                                    conv2_post)

                return chunk

            def everything(_iv=None):
                # Emission strategy: the PE executes in strict emission
                # order (PSUM accumulation groups), while the other engines
                # are re-scheduled by readiness (with a small wait-queue
                # bypass).  The program below is one token list: "dwq"/"dx"
                # tokens define the serial-DMA device order, "c1"/"back"
                # tokens the PE order, "sgn"/"pm" the vector-engine queue
                # positions.  x transfers interleave into the weight DMA
                # stream so neither conv1 (x-gated) nor conv2 (w2-prep-
                # gated) starves the PE in the transition window.
                xts = {}
                c1s, backs, sts = {}, {}, {}

                def create(n):
                    if n not in c1s:
                        mode = half_mode if n in half_imgs else "act"
                        st, fn = make_c1(n, basign(xts.pop(n), mode))
                        sts[n], c1s[n] = st, fn

                def emit(kind, n, c):
                    # lazy creation keeps pool-buffer reuse correct: a
                    # buffer's next writer must be emitted after its
                    # previous readers
                    if kind == "c1":
                        create(n)
                        c1s[n](c)
                    else:
                        if n not in backs:
                            backs[n] = make_back(
                                n, sts[n], last=(n >= IMGS - 2)
                            )
                        backs[n](c)

                warmup()

                if not pipelined:
                    prep_bn(1)
                    prep_bn(2)
                    prep_dma(1, 0)
                    prep_dma(1, 1)
                    prep_math(1, 0)
                    prep_math(1, 1)
                    prep_dma(2, 0)
                    prep_dma(2, 1)
                    prep_math(2, 0)
                    prep_math(2, 1)
                    for n in range(IMGS):
                        xts[n] = xload(n)
                    for n in range(IMGS):
                        for kind in ("c1", "back"):
                            emit(kind, n, 0)
                            emit(kind, n, 1)
                    return

                if prog_variant == 0:
                    # replicates the tuned baseline order (+warmup/gates)
                    prog = [
                        ("dwq", 1, 0, 0), ("dwq", 1, 0, 1),
                        ("dwq", 1, 1, 0), ("dwq", 1, 1, 1),
                        ("wgw", 1, 0, 0), ("wgw", 1, 0, 1),
                        ("wgw", 1, 1, 0), ("wgw", 1, 1, 1),
                        ("bn", 1, 0), ("bn", 2, 0),
                        ("pm", 1, 0), ("wgs", 1, 0), ("pm", 1, 1),
                        ("dx", 0, 0), ("dx", 0, 1, "s"), ("wgx", 0),
                        ("dx", 1, 0),
                        (("dx", 1, 1, "s") if split_x1 else ("dx", 1, 1)),
                    ]
                    if reorder_x2:
                        prog += [
                            ("dx", 2, 0),
                            ("dwq", 2, 0, 0), ("dwq", 2, 0, 1),
                            ("dx", 2, 1),
                        ]
                    else:
                        prog += [
                            ("dwq", 2, 0, 0), ("dwq", 2, 0, 1),
                            ("dx", 2, 0), ("dx", 2, 1, "s"),
                        ]
                    prog += [
                        ("dwq", 2, 1, 0), ("dwq", 2, 1, 1),
                        ("dx", 3, 0), ("dx", 3, 1, "s"),
                        ("c1", 0, 0), ("c1", 1, 0), ("c1", 0, 1),
                        ("sgn", 2),
                        ("c1", 1, 1), ("pm", 2, 0),
                        ("c1", 2, 0),
                        ("sgn", 3),
                        ("c1", 2, 1), ("pm", 2, 1),
                    ]
                    # steady state: backs lead by one chunk so ready
                    # conv2 work covers every ba1(n)/w2-encode wait; c1(3,1)
                    # covers the w2c1-encode latency for back(0,1)
                    prog += [
                        ("dx", 4, 0), ("dx", 4, 1),
                        ("back", 0, 0), ("back", 1, 0),
                        ("c1", 3, 0), ("c1", 3, 1), ("back", 0, 1),
                        ("dx", 5, 0), ("dx", 5, 1),
                        ("back", 1, 1), ("c1", 4, 0), ("back", 2, 0), ("c1", 4, 1),
                        ("dx", 6, 0), ("dx", 6, 1),
                        ("back", 2, 1), ("c1", 5, 0), ("back", 3, 0), ("c1", 5, 1),
                        ("dx", 7, 0), ("dx", 7, 1),
                        ("back", 3, 1), ("c1", 6, 0), ("back", 4, 0), ("c1", 6, 1),
                        ("back", 4, 1), ("c1", 7, 0), ("back", 5, 0), ("c1", 7, 1),
                        ("back", 5, 1), ("back", 6, 0), ("back", 7, 0),
                        ("back", 6, 1), ("back", 7, 1),
                    ]
                else:
                    prog = [
                        ("dwq", 1, 0, 0), ("dwq", 1, 0, 1),
                        ("dwq", 1, 1, 0), ("dwq", 1, 1, 1),
                        ("wgw", 1, 0, 0), ("wgw", 1, 0, 1),
                        ("wgw", 1, 1, 0), ("wgw", 1, 1, 1),
                        ("bn", 1, 0), ("bn", 2, 0),
                        ("pm", 1, 0), ("wgs", 1, 0), ("pm", 1, 1),
                        ("dx", 0, 0), ("dx", 0, 1), ("wgx", 0), ("sgn", 0),
                        ("dx", 1, 0), ("dwq", 2, 0, 0),
                        ("dx", 1, 1), ("dwq", 2, 0, 1),
                        ("c1", 0, 0), ("c1", 0, 1),
                        ("sgn", 1),
                        ("dx", 2, 0), ("dx", 2, 1),
                        ("c1", 1, 0), ("c1", 1, 1),
                        ("pm", 2, 0),
                        ("sgn", 2),
                        ("dwq", 2, 1, 0), ("dx", 3, 0),
                        ("dwq", 2, 1, 1), ("dx", 3, 1),
                        ("c1", 2, 0), ("c1", 2, 1),
                        ("pm", 2, 1),
                        ("sgn", 3),
                        ("dx", 4, 0), ("dx", 4, 1),
                    ]
                    for n in range(IMGS - 3):
                        prog += [
                            ("back", n, 0), ("c1", n + 3, 0),
                            ("back", n, 1), ("c1", n + 3, 1),
                        ]
                        if n + 5 < IMGS:
                            prog += [("dx", n + 5, 0), ("dx", n + 5, 1)]
                        if n + 4 < IMGS:
                            prog += [("sgn", n + 4)]
                    for n in range(IMGS - 3, IMGS):
                        prog += [("back", n, 0), ("back", n, 1)]

                for tok in prog:
                    kind = tok[0]
                    if kind == "dwq":
                        prep_dma_q(tok[1], tok[2], tok[3])
                    elif kind == "dx":
                        n, b = tok[1], tok[2]
                        if n not in xts:
                            xts[n] = {}
                        xts[n][b] = xpool.tile(
                            [P, PIX], FP, tag=f"x{b}", name=f"x{b}"
                        )
                        if len(tok) > 3:
                            # split into two half transfers so the sign of
                            # the first half starts one transfer earlier
                            hp = PIX // 2
                            for q in range(2):
                                nc.sync.dma_start(
                                    out=xts[n][b][:, q * hp : (q + 1) * hp],
                                    in_=x_ext[n, b][:, q * hp : (q + 1) * hp],
                                )
                            xts[n]["split", b] = True
                        else:
                            nc.sync.dma_start(out=xts[n][b], in_=x_ext[n, b])
                    elif kind == "wgw":
                        warm_gate(
                            wstg[(tok[1], tok[2])][:, tok[3]].rearrange(
                                "p a b -> p (a b)"
                            )
                        )
                    elif kind == "wgs":
                        warm_gate(wdr[(tok[1], 0, tok[2])][:, 0])
                    elif kind == "wgx":
                        warm_gate(xts[tok[1]][0])
                    elif kind == "bn":
                        prep_bn(tok[1])
                    elif kind == "sgn":
                        create(tok[1])
                    elif kind == "pm":
                        prep_math(tok[1], tok[2])
                    else:
                        emit(kind, tok[1], tok[2])

            if loop_r is None:
                everything()
            else:
                with tc.For_i(0, loop_r, 1) as iv:
                    everything(iv)

    nc.compile()
    return nc


_NC_CACHE = None


def _get_program():
    global _NC_CACHE
    if _NC_CACHE is None:
        _NC_CACHE = build_program()
    return _NC_CACHE


def make_in_maps(inputs):
    x = np.ascontiguousarray(inputs["x"], dtype=np.float32).reshape(
        N_CORES, IMGS, CH, P, PIX
    )
    shared = {}
    for i in (1, 2):
        # [co, ci, kh, kw] -> [ci, co, tap] -> chunked [CH, P, 256, 9]
        shared[f"conv{i}_w"] = np.ascontiguousarray(
            np.asarray(inputs[f"conv{i}_w"], dtype=np.float32)
            .reshape(CH * P, CH * P, 9)
            .transpose(1, 0, 2)
        ).reshape(CH, P, CH * P, 9)
        packed = np.stack(
            [
                np.asarray(inputs[f"alpha{i}"], dtype=np.float32).reshape(CH * P),
                np.asarray(inputs[f"bn{i}_gamma"], dtype=np.float32),
                np.asarray(inputs[f"bn{i}_beta"], dtype=np.float32),
                np.asarray(inputs[f"bn{i}_mean"], dtype=np.float32),
                np.asarray(inputs[f"bn{i}_var"], dtype=np.float32),
            ],
            axis=-1,
        ).reshape(CH, P, 5)
        # -> [co%128, co_chunk*5 + k]
        shared[f"bn{i}_all"] = np.ascontiguousarray(
            packed.transpose(1, 0, 2)
        ).reshape(P, CH * 5)
    return [{"x": x[c], **shared} for c in range(N_CORES)]


def kernel(**inputs):
    nc = _get_program()
    in_maps = make_in_maps(inputs)
    res = run_bass_kernel_spmd(nc, in_maps, list(range(N_CORES)))
    out = np.stack(
        [np.asarray(res.results[c]["out"]).astype(np.float32) for c in range(N_CORES)]
    )
    return out.reshape(N_CORES * IMGS, CH * P, H, W)



# revision 54
# speedup vs baseline: 1.0004x; 1.0004x over previous
"""Trainium2 Bass kernel for a binarized (1w1a) ResNet BasicBlock.

Computation (eval mode):
    out = hardtanh(bn2(conv2(sign(out1)) * alpha2) + x)
    out1 = hardtanh(bn1(conv1(sign(x)) * alpha1))
with conv_k a 3x3 stride-1 pad-1 conv whose weights are binarized to
sign(w - rowmean(w)).  Binary operands are exact in fp8e4m3 and PSUM
accumulation is fp32, so the conv arithmetic is exact.

Layout / strategy:
 - Data-parallel over batch N=64 -> 8 images per NeuronCore.
 - Channels 256 = 2 chunks of 128 partitions.  DoubleRow fp8 matmuls
   contract over both chunks at once (K=256): lhsT [128, 2, 128],
   rhs [128, 2, rows, cols].
 - Each conv = 9 shifted matmuls (3x3 taps) accumulated into PSUM per
   (co_chunk, 16-row half).  Boundary taps use reduced row AND column
   ranges (no zero-padding needed at all); the first tap (dh=dw=0)
   covers the full half so start=True clears every has_written bit.
 - Weight prep per (conv, co_chunk), no PE/PSUM involvement: DMA ->
   tap reduce (DVE) -> partition all-reduce (GPSIMD) -> compare vs
   mean.  conv1 weights become {-.5,+.5} via fused is_ge ops on
   DVE/Pool (s1 doubled); conv2 chunk-0 weights {-1,+1} via subtract +
   ACT Sign, chunk-1 {-.5,+.5} via DVE/Pool is_ge (s2 of that chunk
   doubled) — chunk 1's encode would otherwise queue behind the x
   binarizations + ba2 posts on the saturated ACT and stall back(0,1).
   The conv matmul lhsT reads sgn[:, :, :, t] directly (strided AP,
   no repack copies).
 - conv1 -> bn -> hardtanh -> sign fuses to one ACT op per half:
   ba2 = sign(s1*psum + t1).
 - conv2 epilogue: xp = x + t2 per image-chunk on GPSIMD (readiness-
   gated behind conv1(n) so Pool stays clear in the DMA-bound
   transition window); per half one DVE scalar_tensor_tensor
   v = s2*psum + xp (bf16 out) and one clamp; store bf16 (host
   converts to fp32; 2^-9 rounding << 2e-2 tol).
 - The PE executes in emission order, other engines dispatch by
   readiness (with a 4-deep wait-queue bypass): conv work is emitted at
   chunk granularity in an explicit token program matching dependency
   readiness (x DMA + sign for conv1, weight-prep chains for conv2);
   all DMA on SP HWDGE in first-need order: w1, bn, x0, x1, w2c0, x2,
   w2c1, x3, x4...  The x0/x1 chunk-1 transfers are split in halves so
   their ACT signs (the head's last gate) start one transfer earlier.
 - PE warmup: 16 garbage matmuls at t~1us plus tiny "warm gate" pings
   (one per w1-quarter landing, one on sgn1_c0, one on x0c0) keep the
   PE busy-run alive through the DMA-bound head, so the p-state ramp
   (low->mid->full over ~4.8us of PE busy-run age) is fully burned
   before the first real conv matmul; all conv matmuls run at full
   clock.  The garbage inputs are memset-zero fp8 tiles; the scratch
   PSUM slot is start=True-cleared by later conv groups.
 - Tail: the last two images' conv2 chunk 1 runs with 8-row blocks
   whose epilogues pipeline with the conv; each 16-row half is
   assembled in a persistent bf16 tile and stored with a single DMA
   (more, smaller stores would serialize on the 625ns HWDGE
   descriptor device).
 - Steady state: conv2 (back) chunks lead the conv1 chunks by one
   slot so ready conv2 work covers every ba1(n)-sign and w2-encode
   wait; c1(3,1) covers the w2c1-encode latency for back(0,1).
"""

import numpy as np

import concourse.bass as bass  # noqa: F401  (AP types referenced via APIs)
import concourse.bass_isa as bass_isa
import concourse.mybir as mybir
import concourse.tile as tile
from concourse import bacc
from concourse.bass_utils import run_bass_kernel_spmd

N_CORES = 8
IMGS = 8  # images per core
CH = 2  # channel chunks of 128
P = 128
H = 32
W = 32
PIX = H * W
BASTRIDE = PIX + 16  # slack so boundary-tap AP slicing stays in bounds
HALF = 16  # rows per output half (psum free = 16*32 = 512)
EPS = 1e-5
FP = mybir.dt.float32
BF = mybir.dt.bfloat16
F8 = mybir.dt.float8e4
AF = mybir.ActivationFunctionType
DR = mybir.MatmulPerfMode.DoubleRow

# dh=dw=0 first: the first matmul of each accumulation group must cover
# the full half (start=True clears the whole bank's has_written bits)
TAPS = [4, 3, 5, 1, 7, 0, 2, 6, 8]

# superseded by the xpadd gate (kept for experiments): images whose xp
# emission is deferred to back(n-2)
XP_DEFER = ()


def _tap_geom(hs, t, nr):
    """Valid local output rows [r0,r1) and cols [c0,c1) for tap t."""
    dh, dw = t // 3 - 1, t % 3 - 1
    r0 = max(0, -(hs + dh))
    r1 = min(nr, H - hs - dh)
    c0 = max(0, -dw)
    c1 = min(W, W - dw)
    return dh, dw, r0, r1, c0, c1


def _conv_chunk(nc, pspool, wdr, conv, ba, c, consumer, rblock=HALF):
    """One co-chunk of a binarized 3x3 conv over one image (DoubleRow fp8).

    ba: [P, 2, BASTRIDE] fp8 tile; chunk i at [:, i, 0:PIX], rows at
    stride W (no padding columns - boundary taps trim rows/cols).
    consumer(c, h, hs, nr, ps) reads the [P, nr*W] fp32 PSUM tile.
    rblock < HALF splits the accumulation into smaller row blocks so the
    epilogue pipelines with the conv (used to shorten the final tail).
    """
    for h in range(H // rblock):
        hs = h * rblock
        ps = pspool.tile([P, rblock * W], FP, tag="ps", name=f"ps{conv}_{c}_{h}")
        psv = ps.rearrange("p (r w) -> p r w", r=rblock)
        for it, t in enumerate(TAPS):
            dh, dw, r0, r1, c0, c1 = _tap_geom(hs, t, rblock)
            s = (hs + r0 + dh) * W + c0 + dw
            rhs = ba[:, :, s : s + (r1 - r0) * W].rearrange(
                "p i (r w) -> p i r w", r=r1 - r0
            )[:, :, :, 0 : c1 - c0]
            nc.tensor.matmul(
                psv[:, r0:r1, c0:c1],
                wdr[(conv, t, c)],
                rhs,
                start=(it == 0),
                stop=(it == 8),
                perf_mode=DR,
                skip_group_check=True,
            )
        consumer(c, h, hs, rblock, ps)


def build_program(loop_r=None, pipelined=True, loop_scope="images",
                  half_imgs=frozenset(), tail_rb=8, prog_variant=0,
                  w2_enc="ts1", tail_rb0=False, split_x1=True,
                  reorder_x2=False, half_mode="half"):
    """loop_r: if set, wrap the workload in For_i(0, loop_r) -
    benchmarking only (re-processes the same images each iteration).
    loop_scope: 'images' loops only the image pipeline; 'all' also loops
    weight prep + BN constant computation (approximates single-shot cost)."""
    nc = bacc.Bacc("TRN2", target_bir_lowering=False, debug=False, num_devices=N_CORES)

    x_ext = nc.dram_tensor("x", [IMGS, CH, P, PIX], FP, kind="ExternalInput").ap()
    w_ext = {}
    bn_ext = {}
    for i in (1, 2):
        # transposed layout from host: [ci_chunk, ci%128, co, tap]
        w_ext[i] = nc.dram_tensor(
            f"conv{i}_w", [CH, P, CH * P, 9], FP, kind="ExternalInput"
        ).ap()
        # packed BN+alpha: [co%128, co_chunk*5 + {alpha,gamma,beta,mean,var}]
        bn_ext[i] = nc.dram_tensor(
            f"bn{i}_all", [P, CH * 5], FP, kind="ExternalInput"
        ).ap()
    out_ext = nc.dram_tensor("out", [IMGS, CH, P, PIX], BF, kind="ExternalOutput").ap()

    with tile.TileContext(nc) as tc:
        from contextlib import ExitStack

        with ExitStack() as ctx:
            singles = ctx.enter_context(tc.tile_pool(name="singles", bufs=1))
            wpool = ctx.enter_context(tc.tile_pool(name="wpool", bufs=1))
            wstage = ctx.enter_context(tc.tile_pool(name="wstage", bufs=2))
            xpool = ctx.enter_context(tc.tile_pool(name="xpool", bufs=5))
            xppool = ctx.enter_context(tc.tile_pool(name="xppool", bufs=5))
            bapool = ctx.enter_context(tc.tile_pool(name="bapool", bufs=5))
            vpool = ctx.enter_context(tc.tile_pool(name="vpool", bufs=6))
            pspool = ctx.enter_context(tc.tile_pool(name="psum", bufs=8, space="PSUM"))

            eps_t = singles.tile([P, 1], FP)
            nc.vector.memset(eps_t, EPS)

            def warmup(n_mm=16, width=512):
                """Emit garbage matmuls at t~1us so the sim's PE p-state
                ramp (low->mid->full over ~4.8us from the start of the
                current PE busy run) is burned while the PE would be idle
                waiting on weight DMA+prep.  Together with warm_gate()
                pings this keeps the PE "run" alive until the first real
                conv matmul, which then runs at full speed.  Inputs are a
                memset fp8 tile (products are 0); the PSUM scratch uses
                the shared "ps" ring and is never read."""
                wz = singles.tile([P, CH, width], F8, tag="wuw", name="wuw")
                nc.gpsimd.memset(wz, 0.0)
                ps = pspool.tile([P, width], FP, tag="ps", name="wups")
                for _ in range(n_mm):
                    nc.tensor.matmul(
                        ps, wz[:, :, 0:P], wz, start=True, stop=True,
                        perf_mode=DR, skip_group_check=True,
                    )

            def warm_gate(gate, fp32=True):
                """One tiny matmul whose rhs/lhsT read `gate` (a [128, >=128]
                fp32 AP, or fp8 with fp32=False): becomes READY when the
                gate tile is written, pinging the PE so its busy-run (and
                thus the p-state ramp credit) survives the head's idle
                stretches between DMA arrivals."""
                ps = pspool.tile([P, 16], FP, tag="ps", name="wgps")
                nc.tensor.matmul(
                    ps, gate[:, 0:P], gate[:, 0:16], start=True, stop=True,
                    skip_group_check=True,
                )

            # ---- weight prep (transposed host layout [ci, co, tap]):
            # binarize vs the per-co mean over (ci, tap).  Pipelined per
            # co-chunk so conv1's first matmuls can start before the whole
            # weight tensor is processed; see prep_math for the engine
            # split.  The conv matmul lhsT reads sgn[:, :, :, t] directly
            # (strided AP).
            wdr = {}
            s_t = {}
            t_t = {}

            wstg = {}

            def prep_dma_q(i, c, b):
                """One quarter (ci-chunk b of co-chunk c) of conv i's raw
                weights.  Quarter-granularity lets x transfers interleave
                into the weight stream on the serial DMA device."""
                if (i, c) not in wstg:
                    wstg[(i, c)] = wstage.tile(
                        [P, CH, P, 9], FP, tag=f"wtraw{c}", name=f"wtraw{i}_{c}"
                    )
                nc.sync.dma_start(
                    out=wstg[(i, c)][:, b], in_=w_ext[i][b][:, c * P : (c + 1) * P]
                )

            def prep_dma(i, c):
                for b in range(CH):
                    prep_dma_q(i, c, b)

            def prep_reduce(i, c):
                # tap-reduce (DVE) of the raw weight tile; emitted for both
                # co-chunks before any encode so the chunk-1 reduces are not
                # queued behind chunk-0's DVE encode ops.
                wT = wstg[(i, c)]
                tap = wstage.tile([P, CH, P], FP, tag=f"tap{c}")
                for b in range(CH):
                    nc.vector.tensor_reduce(
                        out=tap[:, b], in_=wT[:, b], axis=mybir.AxisListType.X,
                        op=mybir.AluOpType.add,
                    )
                wstg[(i, c, "tap")] = tap

            def prep_math(i, c):
                # mean via gpsimd partition all-reduce of the tap sums:
                # no PE matmuls, no PSUM — the PE queue stays pure conv work.
                # Compare w*2304 >= colsum instead of w >= mean (same ulp-level
                # rounding class as an explicit division).
                if (i, c, "tap") not in wstg:
                    prep_reduce(i, c)
                wT = wstg.pop((i, c))
                tap = wstg.pop((i, c, "tap"))
                asum = wstage.tile([P, CH, P], FP, tag=f"asum{c}")
                nc.gpsimd.partition_all_reduce(
                    asum, tap, channels=P, reduce_op=bass_isa.ReduceOp.add
                )
                csum = wstage.tile([P, P], FP, tag=f"csum{c}")
                nc.gpsimd.tensor_tensor(
                    out=csum, in0=asum[:, 0], in1=asum[:, 1], op=mybir.AluOpType.add
                )
                sgn = wpool.tile(
                    [P, CH, P, 9], F8, tag=f"sgn{i}_{c}", name=f"sgn{i}_{c}"
                )
                mean = wstage.tile([P, P], FP, tag=f"mean{c}")
                nc.gpsimd.tensor_scalar_mul(mean, csum, 1.0 / (CH * P * 9))
                if i == 1:
                    # conv1 (head-critical): bw in {-.5,+.5} (s1 is doubled),
                    # avoiding ACT which is busy with the x binarizations.
                    # b0 on DVE: fused (w*2304 >= colsum) then -0.5.
                    nc.vector.scalar_tensor_tensor(
                        out=sgn[:, 0], in0=wT[:, 0], scalar=float(CH * P * 9),
                        in1=csum.to_broadcast([P, P, 9]),
                        op0=mybir.AluOpType.mult, op1=mybir.AluOpType.is_ge,
                    )
                    nc.vector.tensor_scalar(
                        out=sgn[:, 0], in0=sgn[:, 0], scalar1=0.5, scalar2=None,
                        op0=mybir.AluOpType.subtract,
                    )
                    # b1: d = w - mean (sign-exact), fused (d >= 0) - 0.5;
                    # split by co-half across Pool and DVE — the Pool
                    # tensor_tensor runs at 0.42 efficiency and this chain
                    # is the head-critical path
                    d = wstage.tile([P, P, 9], FP, tag=f"d1_{c}")
                    hb = P // 2
                    for q, eng in ((0, nc.gpsimd), (1, nc.vector)):
                        sl = slice(q * hb, (q + 1) * hb)
                        eng.tensor_tensor(
                            out=d[:, sl], in0=wT[:, 1, sl],
                            in1=mean[:, sl].to_broadcast([P, hb, 9]),
                            op=mybir.AluOpType.subtract,
                        )
                        eng.tensor_scalar(
                            out=sgn[:, 1, sl], in0=d[:, sl], scalar1=0.0,
                            scalar2=0.5, op0=mybir.AluOpType.is_ge,
                            op1=mybir.AluOpType.subtract,
                        )
                else:
                    # conv2: d on DVE/Pool, then either sign to +-1 on ACT
                    # (w2_enc="act") or the {-.5,+.5} fused is_ge encode on
                    # DVE/Pool (w2_enc="ts", s2 doubled) to keep ACT free
                    # for the x binarizations + ba2 posts in the congested
                    # transition window.  b1's subtract is co-half split
                    # across Pool+DVE (Pool tensor_tensor runs at 0.42
                    # efficiency; this chain gates back(0))
                    hb = P // 2
                    for b in range(CH):
                        d = wstage.tile([P, P, 9], FP, tag=f"d{b}_{c}")
                        if b == 0:
                            nc.vector.tensor_tensor(
                                out=d, in0=wT[:, b],
                                in1=mean.to_broadcast([P, P, 9]),
                                op=mybir.AluOpType.subtract,
                            )
                        else:
                            for q, eng in ((0, nc.gpsimd), (1, nc.vector)):
                                sl = slice(q * hb, (q + 1) * hb)
                                eng.tensor_tensor(
                                    out=d[:, sl], in0=wT[:, b, sl],
                                    in1=mean[:, sl].to_broadcast([P, hb, 9]),
                                    op=mybir.AluOpType.subtract,
                                )
                        if w2_enc == "act" or (w2_enc == "ts1" and c == 0):
                            nc.scalar.sign(sgn[:, b], d)
                        else:
                            for q, eng in ((0, nc.gpsimd), (1, nc.vector)):
                                sl = slice(q * hb, (q + 1) * hb)
                                eng.tensor_scalar(
                                    out=sgn[:, b, sl], in0=d[:, sl],
                                    scalar1=0.0, scalar2=0.5,
                                    op0=mybir.AluOpType.is_ge,
                                    op1=mybir.AluOpType.subtract,
                                )
                for t in range(9):
                    wdr[(i, t, c)] = sgn[:, :, :, t]

            # ---- BN constants: s = alpha*gamma/sqrt(var+eps),
            #                    t = beta - mean*gamma/sqrt(var+eps)
            def prep_bn(i):
                bn_t = singles.tile([P, CH * 5], FP, tag=f"bn{i}", name=f"bn{i}")
                nc.sync.dma_start(out=bn_t, in_=bn_ext[i])
                for c in range(CH):
                    ld = {
                        nm: bn_t[:, c * 5 + k : c * 5 + k + 1]
                        for k, nm in enumerate(("alpha", "gamma", "beta", "mean", "var"))
                    }
                    std = singles.tile([P, 1], FP, tag=f"std{i}_{c}", name=f"std{i}_{c}")
                    nc.scalar.activation(std, ld["var"], AF.Sqrt, bias=eps_t)
                    g = singles.tile([P, 1], FP, tag=f"g{i}_{c}", name=f"g{i}_{c}")
                    nc.vector.reciprocal(g, std)
                    nc.vector.tensor_mul(g, g, ld["gamma"])
                    s = singles.tile([P, 1], FP, tag=f"s{i}_{c}", name=f"s{i}_{c}")
                    nc.vector.tensor_mul(s, g, ld["alpha"])
                    if i == 1:
                        nc.vector.tensor_add(s, s, s)  # x2: conv1 bw +-0.5
                        s4 = singles.tile(
                            [P, 1], FP, tag=f"s4_{c}", name=f"s4_{c}"
                        )
                        # x4 scale for images whose ba1 is half-encoded
                        nc.vector.tensor_add(s4, s, s)
                        s_t[("1x2", c)] = s4
                    elif w2_enc == "ts" or (w2_enc == "ts1" and c == 1):
                        nc.vector.tensor_add(s, s, s)  # x2: conv2 bw +-0.5
                    tt = singles.tile([P, 1], FP, tag=f"t{i}_{c}", name=f"t{i}_{c}")
                    nc.vector.tensor_mul(tt, g, ld["mean"])
                    nc.vector.tensor_sub(tt, ld["beta"], tt)
                    s_t[(i, c)] = s
                    t_t[(i, c)] = tt

            # ---- per-image stages -------------------------------------
            def xload(n):
                """x(n) DMA triggers on the SP queue.  Image 0's second
                chunk is split in two transfers so its binarization (the
                head's last gate) can start on the first half sooner."""
                xt = {}
                for b in range(CH):
                    xt[b] = xpool.tile([P, PIX], FP, tag=f"x{b}", name=f"x{b}")
                    nc.sync.dma_start(out=xt[b], in_=x_ext[n, b])
                return xt

            def basign(state, mode="act"):
                """ba1(n): +-1 sign on ACT ("act"), or the {-.5,+.5}
                encoding via one fused tensor_scalar per chunk — "half"
                puts both chunks on DVE (plain-ts runs at the 2x_2p rate
                there), "half-mixed" splits DVE/Pool.  conv1's post
                compensates half encodings with a doubled scale."""
                xt = state
                half = mode != "act"
                ba1 = bapool.tile([P, CH, BASTRIDE], F8, tag="ba1", name="ba1")
                for b in range(CH):
                    if half:
                        eng = (
                            nc.gpsimd
                            if (mode == "half-mixed" and b == 1)
                            else nc.vector
                        )
                        eng.tensor_scalar(
                            out=ba1[:, b, 0:PIX], in0=xt[b], scalar1=0.0,
                            scalar2=0.5, op0=mybir.AluOpType.is_ge,
                            op1=mybir.AluOpType.subtract,
                        )
                    elif xt.get(("split", b)):
                        hp = (HALF + 1) * W
                        for sl in (slice(0, hp), slice(hp, PIX)):
                            nc.scalar.sign(
                                ba1[:, b, sl], xt[b][:, sl],
                            )
                    else:
                        nc.scalar.sign(ba1[:, b, 0:PIX], xt[b])
                return xt, ba1, half

            def xpadd(xt, gate=None):
                """xp(n) = x(n) + t2 on GPSIMD (residual + BN shift).

                gate: an AP written late (e.g. a ba2 column).  The t2 scalar
                is routed through a tiny gate op that reads it, so the xp
                ops only become READY once conv1(n) is under way — keeping
                Pool free during the DMA-bound transition window (the
                scheduler dispatches by readiness, not emission order).
                """
                xp = {}
                for b in range(CH):
                    t2b = t_t[(2, b)]
                    if gate is not None:
                        t2l = xppool.tile([P, 1], FP, tag=f"t2l{b}", name=f"t2l{b}")
                        nc.vector.scalar_tensor_tensor(
                            out=t2l, in0=gate, scalar=0.0, in1=t2b,
                            op0=mybir.AluOpType.mult, op1=mybir.AluOpType.add,
                        )
                        t2b = t2l
                    xp[b] = xppool.tile([P, PIX], FP, tag=f"xp{b}", name=f"xp{b}")
                    nc.gpsimd.tensor_scalar_add(xp[b], xt[b], t2b)
                return xp

            def make_c1(n, state):
                """conv1(n) -> ba2(n); returns (shared-state, chunk-emitter)."""
                xt, ba1, half = state
                skey = "1x2" if half else 1
                st = {}

                def chunk(c):
                    first = not st
                    if first:
                        st["ba2"] = bapool.tile(
                            [P, CH, BASTRIDE], F8, tag="ba2", name="ba2"
                        )
                        st["xt"] = xt

                    def conv1_post(cc, h, hs, nr, ps):
                        # ba2 = sign(s1*conv + t1)  (sign(hardtanh(y))==sign(y))
                        nc.scalar.activation(
                            st["ba2"][:, cc, hs * W : hs * W + nr * W],
                            ps,
                            AF.Sign,
                            bias=t_t[(1, cc)],
                            scale=s_t[(skey, cc)],
                        )

                    _conv_chunk(nc, pspool, wdr, 1, ba1, c, conv1_post)
                    if c == 1:
                        # emitted after the second chunk so the gate read
                        # depends on the last ba2 write (see xpadd)
                        st["xp"] = xpadd(
                            xt, gate=st["ba2"][:, 1, PIX - 1 : PIX]
                        )

                return st, chunk

            def make_back(n, st, last=False):
                def chunk(c):
                    if "xp" not in st:
                        st["xp"] = xpadd(st["xt"])
                    def conv2_post(cc, h, hs, nr, ps):
                        # last image's epilogue is the tail: put the clamp of
                        # alternating halves on Pool (GPSIMD cannot read
                        # PSUM, so the stt stays on DVE)
                        eng = nc.gpsimd if (last and (cc + h) % 2) else nc.vector
                        v = vpool.tile([P, nr * W], BF, tag="v", name="v")
                        nc.vector.scalar_tensor_tensor(
                            out=v, in0=ps, scalar=s_t[(2, cc)],
                            in1=st["xp"][cc][:, hs * W : hs * W + nr * W],
                            op0=mybir.AluOpType.mult, op1=mybir.AluOpType.add,
                        )
                        eng.tensor_scalar(
                            out=v, in0=v, scalar1=1.0, scalar2=-1.0,
                            op0=mybir.AluOpType.min, op1=mybir.AluOpType.max,
                        )
                        nc.sync.dma_start(
                            out=out_ext[n, cc][:, hs * W : hs * W + nr * W],
                            in_=v,
                        )

                    if last and (c == 1 or tail_rb0) and tail_rb:
                        # pipeline the final chunk's epilogue with its conv
                        # (4-row blocks) but keep one store per 16-row half:
                        # 8 small stores would serialize on the 625ns HWDGE
                        vh = {}

                        def conv2_post_last(cc, h, hs, nr, ps):
                            q = hs // HALF  # which 16-row half
                            if q not in vh:
                                vh[q] = vpool.tile(
                                    [P, HALF * W], BF, tag="v", name="vlast"
                                )
                            ro = (hs % HALF) * W
                            sl = slice(ro, ro + nr * W)
                            v = vh[q]
                            nc.vector.scalar_tensor_tensor(
                                out=v[:, sl], in0=ps, scalar=s_t[(2, cc)],
                                in1=st["xp"][cc][:, hs * W : hs * W + nr * W],
                                op0=mybir.AluOpType.mult,
                                op1=mybir.AluOpType.add,
                            )
                            eng = nc.gpsimd if h % 2 else nc.vector
                            eng.tensor_scalar(
                                out=v[:, sl], in0=v[:, sl], scalar1=1.0,
                                scalar2=-1.0, op0=mybir.AluOpType.min,
                                op1=mybir.AluOpType.max,
                            )
                            if hs + nr == (q + 1) * HALF:
                                nc.sync.dma_start(
                                    out=out_ext[n, cc][
                                        :, q * HALF * W : (q + 1) * HALF * W
                                    ],
                                    in_=v,
                                )

                        _conv_chunk(nc, pspool, wdr, 2, st["ba2"], c,
                                    conv2_post_last, rblock=tail_rb)
                    else:
                        _conv_chunk(nc, pspool, wdr, 2, st["ba2"], c,
                                    conv2_post)

                return chunk

            def everything(_iv=None):
                # Emission strategy: the PE executes in strict emission
                # order (PSUM accumulation groups), while the other engines
                # are re-scheduled by readiness (with a small wait-queue
                # bypass).  The program below is one token list: "dwq"/"dx"
                # tokens define the serial-DMA device order, "c1"/"back"
                # tokens the PE order, "sgn"/"pm" the vector-engine queue
                # positions.  x transfers interleave into the weight DMA
                # stream so neither conv1 (x-gated) nor conv2 (w2-prep-
                # gated) starves the PE in the transition window.
                xts = {}
                c1s, backs, sts = {}, {}, {}

                def create(n):
                    if n not in c1s:
                        mode = half_mode if n in half_imgs else "act"
                        st, fn = make_c1(n, basign(xts.pop(n), mode))
                        sts[n], c1s[n] = st, fn

                def emit(kind, n, c):
                    # lazy creation keeps pool-buffer reuse correct: a
                    # buffer's next writer must be emitted after its
                    # previous readers
                    if kind == "c1":
                        create(n)
                        c1s[n](c)
                    else:
                        if n not in backs:
                            backs[n] = make_back(
                                n, sts[n], last=(n >= IMGS - 2)
                            )
                        backs[n](c)

                warmup()

                if not pipelined:
                    prep_bn(1)
                    prep_bn(2)
                    prep_dma(1, 0)
                    prep_dma(1, 1)
                    prep_math(1, 0)
                    prep_math(1, 1)
                    prep_dma(2, 0)
                    prep_dma(2, 1)
                    prep_math(2, 0)
                    prep_math(2, 1)
                    for n in range(IMGS):
                        xts[n] = xload(n)
                    for n in range(IMGS):
                        for kind in ("c1", "back"):
                            emit(kind, n, 0)
                            emit(kind, n, 1)
                    return

                if prog_variant == 0:
                    # replicates the tuned baseline order (+warmup/gates)
                    prog = [
                        ("dwq", 1, 0, 0), ("dwq", 1, 0, 1),
                        ("dwq", 1, 1, 0), ("dwq", 1, 1, 1),
                        ("wgw", 1, 0, 0), ("wgw", 1, 0, 1),
                        ("wgw", 1, 1, 0), ("wgw", 1, 1, 1),
                        ("bn", 1, 0), ("bn", 2, 0),
                        ("pm", 1, 0), ("wgs", 1, 0), ("pm", 1, 1),
                        ("dx", 0, 0), ("dx", 0, 1, "s"), ("wgx", 0),
                        ("dx", 1, 0),
                        (("dx", 1, 1, "s") if split_x1 else ("dx", 1, 1)),
                    ]
                    if reorder_x2:
                        prog += [
                            ("dx", 2, 0),
                            ("dwq", 2, 0, 0), ("dwq", 2, 0, 1),
                            ("dx", 2, 1),
                        ]
                    else:
                        prog += [
                            ("dwq", 2, 0, 0), ("dwq", 2, 0, 1),
                            ("dx", 2, 0), ("dx", 2, 1, "s"),
                        ]
                    prog += [
                        ("dwq", 2, 1, 0), ("dwq", 2, 1, 1),
                        ("dx", 3, 0), ("dx", 3, 1, "s"),
                        ("c1", 0, 0), ("c1", 1, 0), ("c1", 0, 1),
                        ("sgn", 2),
                        ("c1", 1, 1), ("pm", 2, 0),
                        ("c1", 2, 0),
                        ("sgn", 3),
                        ("c1", 2, 1), ("pm", 2, 1),
                    ]
                    # steady state: backs lead by one chunk so ready
                    # conv2 work covers every ba1(n)/w2-encode wait; c1(3,1)
                    # covers the w2c1-encode latency for back(0,1)
                    prog += [
                        ("dx", 4, 0), ("dx", 4, 1),
                        ("back", 0, 0), ("back", 1, 0),
                        ("c1", 3, 0), ("c1", 3, 1), ("back", 0, 1),
                        ("dx", 5, 0), ("dx", 5, 1),
                        ("back", 1, 1), ("c1", 4, 0), ("back", 2, 0), ("c1", 4, 1),
                        ("dx", 6, 0), ("dx", 6, 1),
                        ("back", 2, 1), ("c1", 5, 0), ("back", 3, 0), ("c1", 5, 1),
                        ("dx", 7, 0), ("dx", 7, 1),
                        ("back", 3, 1), ("c1", 6, 0), ("back", 4, 0), ("c1", 6, 1),
                        ("back", 4, 1), ("c1", 7, 0), ("back", 5, 0), ("c1", 7, 1),
                        ("back", 5, 1), ("back", 6, 0), ("back", 7, 0),
                        ("back", 6, 1), ("back", 7, 1),
                    ]
                else:
                    prog = [
                        ("dwq", 1, 0, 0), ("dwq", 1, 0, 1),
                        ("dwq", 1, 1, 0), ("dwq", 1, 1, 1),
                        ("wgw", 1, 0, 0), ("wgw", 1, 0, 1),
                        ("wgw", 1, 1, 0), ("wgw", 1, 1, 1),
                        ("bn", 1, 0), ("bn", 2, 0),
                        ("pm", 1, 0), ("wgs", 1, 0), ("pm", 1, 1),
                        ("dx", 0, 0), ("dx", 0, 1), ("wgx", 0), ("sgn", 0),
                        ("dx", 1, 0), ("dwq", 2, 0, 0),
                        ("dx", 1, 1), ("dwq", 2, 0, 1),
                        ("c1", 0, 0), ("c1", 0, 1),
                        ("sgn", 1),
                        ("dx", 2, 0), ("dx", 2, 1),
                        ("c1", 1, 0), ("c1", 1, 1),
                        ("pm", 2, 0),
                        ("sgn", 2),
                        ("dwq", 2, 1, 0), ("dx", 3, 0),
                        ("dwq", 2, 1, 1), ("dx", 3, 1),
                        ("c1", 2, 0), ("c1", 2, 1),
                        ("pm", 2, 1),
                        ("sgn", 3),
                        ("dx", 4, 0), ("dx", 4, 1),
                    ]
                    for n in range(IMGS - 3):
                        prog += [
                            ("back", n, 0), ("c1", n + 3, 0),
                            ("back", n, 1), ("c1", n + 3, 1),
                        ]
                        if n + 5 < IMGS:
                            prog += [("dx", n + 5, 0), ("dx", n + 5, 1)]
                        if n + 4 < IMGS:
                            prog += [("sgn", n + 4)]
                    for n in range(IMGS - 3, IMGS):
                        prog += [("back", n, 0), ("back", n, 1)]

                for tok in prog:
                    kind = tok[0]
                    if kind == "dwq":
                        prep_dma_q(tok[1], tok[2], tok[3])
                    elif kind == "dx":
                        n, b = tok[1], tok[2]
                        if n not in xts:
                            xts[n] = {}
                        xts[n][b] = xpool.tile(
                            [P, PIX], FP, tag=f"x{b}", name=f"x{b}"
                        )
                        if len(tok) > 3:
                            # split into two half transfers so the sign of
                            # the first half starts one transfer earlier
                            # split at row 17 (not 16): conv h0's dh=+1
                            # taps read ba row 16, so a 512-split makes h0
                            # cross into the second half-sign
                            hp = (HALF + 1) * W
                            for sl in (slice(0, hp), slice(hp, PIX)):
                                nc.sync.dma_start(
                                    out=xts[n][b][:, sl],
                                    in_=x_ext[n, b][:, sl],
                                )
                            xts[n]["split", b] = True
                        else:
                            nc.sync.dma_start(out=xts[n][b], in_=x_ext[n, b])
                    elif kind == "wgw":
                        warm_gate(
                            wstg[(tok[1], tok[2])][:, tok[3]].rearrange(
                                "p a b -> p (a b)"
                            )
                        )
                    elif kind == "wgs":
                        warm_gate(wdr[(tok[1], 0, tok[2])][:, 0])
                    elif kind == "wgx":
                        warm_gate(xts[tok[1]][0])
                    elif kind == "bn":
                        prep_bn(tok[1])
                    elif kind == "sgn":
                        create(tok[1])
                    elif kind == "pm":
                        prep_math(tok[1], tok[2])
                    else:
                        emit(kind, tok[1], tok[2])

            if loop_r is None:
                everything()
            else:
                with tc.For_i(0, loop_r, 1) as iv:
                    everything(iv)

    nc.compile()
    return nc


_NC_CACHE = None


def _get_program():
    global _NC_CACHE
    if _NC_CACHE is None:
        _NC_CACHE = build_program()
    return _NC_CACHE


def make_in_maps(inputs):
    x = np.ascontiguousarray(inputs["x"], dtype=np.float32).reshape(
        N_CORES, IMGS, CH, P, PIX
    )
    shared = {}
    for i in (1, 2):
        # [co, ci, kh, kw] -> [ci, co, tap] -> chunked [CH, P, 256, 9]
        shared[f"conv{i}_w"] = np.ascontiguousarray(
            np.asarray(inputs[f"conv{i}_w"], dtype=np.float32)
            .reshape(CH * P, CH * P, 9)
            .transpose(1, 0, 2)
        ).reshape(CH, P, CH * P, 9)
        packed = np.stack(
            [
                np.asarray(inputs[f"alpha{i}"], dtype=np.float32).reshape(CH * P),
                np.asarray(inputs[f"bn{i}_gamma"], dtype=np.float32),
                np.asarray(inputs[f"bn{i}_beta"], dtype=np.float32),
                np.asarray(inputs[f"bn{i}_mean"], dtype=np.float32),
                np.asarray(inputs[f"bn{i}_var"], dtype=np.float32),
            ],
            axis=-1,
        ).reshape(CH, P, 5)
        # -> [co%128, co_chunk*5 + k]
        shared[f"bn{i}_all"] = np.ascontiguousarray(
            packed.transpose(1, 0, 2)
        ).reshape(P, CH * 5)
    return [{"x": x[c], **shared} for c in range(N_CORES)]


def kernel(**inputs):
    nc = _get_program()
    in_maps = make_in_maps(inputs)
    res = run_bass_kernel_spmd(nc, in_maps, list(range(N_CORES)))
    out = np.stack(
        [np.asarray(res.results[c]["out"]).astype(np.float32) for c in range(N_CORES)]
    )
    return out.reshape(N_CORES * IMGS, CH * P, H, W)



# revision 55
# speedup vs baseline: 1.0041x; 1.0037x over previous
"""Trainium2 Bass kernel for a binarized (1w1a) ResNet BasicBlock.

Computation (eval mode):
    out = hardtanh(bn2(conv2(sign(out1)) * alpha2) + x)
    out1 = hardtanh(bn1(conv1(sign(x)) * alpha1))
with conv_k a 3x3 stride-1 pad-1 conv whose weights are binarized to
sign(w - rowmean(w)).  Binary operands are exact in fp8e4m3 and PSUM
accumulation is fp32, so the conv arithmetic is exact.

Layout / strategy:
 - Data-parallel over batch N=64 -> 8 images per NeuronCore.
 - Channels 256 = 2 chunks of 128 partitions.  DoubleRow fp8 matmuls
   contract over both chunks at once (K=256): lhsT [128, 2, 128],
   rhs [128, 2, rows, cols].
 - Each conv = 9 shifted matmuls (3x3 taps) accumulated into PSUM per
   (co_chunk, 16-row half).  Boundary taps use reduced row AND column
   ranges (no zero-padding needed at all); the first tap (dh=dw=0)
   covers the full half so start=True clears every has_written bit.
 - Weight prep per (conv, co_chunk), no PE/PSUM involvement: DMA ->
   tap reduce (DVE) -> partition all-reduce (GPSIMD) -> compare vs
   mean.  conv1 weights become {-.5,+.5} via fused is_ge ops on
   DVE/Pool (s1 doubled); conv2 chunk-0 weights {-1,+1} via subtract +
   ACT Sign, chunk-1 {-.5,+.5} via DVE/Pool is_ge (s2 of that chunk
   doubled) — chunk 1's encode would otherwise queue behind the x
   binarizations + ba2 posts on the saturated ACT and stall back(0,1).
   The conv matmul lhsT reads sgn[:, :, :, t] directly (strided AP,
   no repack copies).
 - conv1 -> bn -> hardtanh -> sign fuses to one ACT op per half:
   ba2 = sign(s1*psum + t1).
 - conv2 epilogue: xp = x + t2 per image-chunk on GPSIMD (readiness-
   gated behind conv1(n) so Pool stays clear in the DMA-bound
   transition window); per half one DVE scalar_tensor_tensor
   v = s2*psum + xp (bf16 out) and one clamp; store bf16 (host
   converts to fp32; 2^-9 rounding << 2e-2 tol).
 - The PE executes in emission order, other engines dispatch by
   readiness (with a 4-deep wait-queue bypass): conv work is emitted at
   chunk granularity in an explicit token program matching dependency
   readiness (x DMA + sign for conv1, weight-prep chains for conv2);
   all DMA on SP HWDGE in first-need order: w1, bn, x0, x1, w2c0, x2,
   w2c1, x3, x4...  The x0/x1 chunk-1 transfers are split in halves so
   their ACT signs (the head's last gate) start one transfer earlier.
 - PE warmup: 16 garbage matmuls at t~1us plus tiny "warm gate" pings
   (one per w1-quarter landing, one on sgn1_c0, one on x0c0) keep the
   PE busy-run alive through the DMA-bound head, so the p-state ramp
   (low->mid->full over ~4.8us of PE busy-run age) is fully burned
   before the first real conv matmul; all conv matmuls run at full
   clock.  The garbage inputs are memset-zero fp8 tiles; the scratch
   PSUM slot is start=True-cleared by later conv groups.
 - Tail: the last two images' conv2 chunk 1 runs with 8-row blocks
   whose epilogues pipeline with the conv; each 16-row half is
   assembled in a persistent bf16 tile and stored with a single DMA
   (more, smaller stores would serialize on the 625ns HWDGE
   descriptor device).
 - Steady state: conv2 (back) chunks lead the conv1 chunks by one
   slot so ready conv2 work covers every ba1(n)-sign and w2-encode
   wait; c1(3,1) covers the w2c1-encode latency for back(0,1).
"""

import numpy as np

import concourse.bass as bass  # noqa: F401  (AP types referenced via APIs)
import concourse.bass_isa as bass_isa
import concourse.mybir as mybir
import concourse.tile as tile
from concourse import bacc
from concourse.bass_utils import run_bass_kernel_spmd

N_CORES = 8
IMGS = 8  # images per core
CH = 2  # channel chunks of 128
P = 128
H = 32
W = 32
PIX = H * W
BASTRIDE = PIX + 16  # slack so boundary-tap AP slicing stays in bounds
HALF = 16  # rows per output half (psum free = 16*32 = 512)
EPS = 1e-5
FP = mybir.dt.float32
BF = mybir.dt.bfloat16
F8 = mybir.dt.float8e4
AF = mybir.ActivationFunctionType
DR = mybir.MatmulPerfMode.DoubleRow

# dh=dw=0 first: the first matmul of each accumulation group must cover
# the full half (start=True clears the whole bank's has_written bits)
TAPS = [4, 3, 5, 1, 7, 0, 2, 6, 8]

# superseded by the xpadd gate (kept for experiments): images whose xp
# emission is deferred to back(n-2)
XP_DEFER = ()


def _tap_geom(hs, t, nr):
    """Valid local output rows [r0,r1) and cols [c0,c1) for tap t."""
    dh, dw = t // 3 - 1, t % 3 - 1
    r0 = max(0, -(hs + dh))
    r1 = min(nr, H - hs - dh)
    c0 = max(0, -dw)
    c1 = min(W, W - dw)
    return dh, dw, r0, r1, c0, c1


def _conv_chunk(nc, pspool, wdr, conv, ba, c, consumer, rblock=HALF):
    """One co-chunk of a binarized 3x3 conv over one image (DoubleRow fp8).

    ba: [P, 2, BASTRIDE] fp8 tile; chunk i at [:, i, 0:PIX], rows at
    stride W (no padding columns - boundary taps trim rows/cols).
    consumer(c, h, hs, nr, ps) reads the [P, nr*W] fp32 PSUM tile.
    rblock < HALF splits the accumulation into smaller row blocks so the
    epilogue pipelines with the conv (used to shorten the final tail).
    """
    for h in range(H // rblock):
        hs = h * rblock
        ps = pspool.tile([P, rblock * W], FP, tag="ps", name=f"ps{conv}_{c}_{h}")
        psv = ps.rearrange("p (r w) -> p r w", r=rblock)
        for it, t in enumerate(TAPS):
            dh, dw, r0, r1, c0, c1 = _tap_geom(hs, t, rblock)
            s = (hs + r0 + dh) * W + c0 + dw
            rhs = ba[:, :, s : s + (r1 - r0) * W].rearrange(
                "p i (r w) -> p i r w", r=r1 - r0
            )[:, :, :, 0 : c1 - c0]
            nc.tensor.matmul(
                psv[:, r0:r1, c0:c1],
                wdr[(conv, t, c)],
                rhs,
                start=(it == 0),
                stop=(it == 8),
                perf_mode=DR,
                skip_group_check=True,
            )
        consumer(c, h, hs, rblock, ps)


def build_program(loop_r=None, pipelined=True, loop_scope="images",
                  half_imgs=frozenset(), tail_rb=8, prog_variant=0,
                  w2_enc="ts1", tail_rb0=False, split_x1=True,
                  reorder_x2=False, half_mode="half"):
    """loop_r: if set, wrap the workload in For_i(0, loop_r) -
    benchmarking only (re-processes the same images each iteration).
    loop_scope: 'images' loops only the image pipeline; 'all' also loops
    weight prep + BN constant computation (approximates single-shot cost)."""
    nc = bacc.Bacc("TRN2", target_bir_lowering=False, debug=False, num_devices=N_CORES)

    x_ext = nc.dram_tensor("x", [IMGS, CH, P, PIX], FP, kind="ExternalInput").ap()
    w_ext = {}
    bn_ext = {}
    for i in (1, 2):
        # transposed layout from host: [ci_chunk, ci%128, co, tap]
        w_ext[i] = nc.dram_tensor(
            f"conv{i}_w", [CH, P, CH * P, 9], FP, kind="ExternalInput"
        ).ap()
        # packed BN+alpha: [co%128, co_chunk*5 + {alpha,gamma,beta,mean,var}]
        bn_ext[i] = nc.dram_tensor(
            f"bn{i}_all", [P, CH * 5], FP, kind="ExternalInput"
        ).ap()
    out_ext = nc.dram_tensor("out", [IMGS, CH, P, PIX], BF, kind="ExternalOutput").ap()

    with tile.TileContext(nc) as tc:
        from contextlib import ExitStack

        with ExitStack() as ctx:
            singles = ctx.enter_context(tc.tile_pool(name="singles", bufs=1))
            wpool = ctx.enter_context(tc.tile_pool(name="wpool", bufs=1))
            wstage = ctx.enter_context(tc.tile_pool(name="wstage", bufs=2))
            xpool = ctx.enter_context(tc.tile_pool(name="xpool", bufs=5))
            xppool = ctx.enter_context(tc.tile_pool(name="xppool", bufs=5))
            bapool = ctx.enter_context(tc.tile_pool(name="bapool", bufs=5))
            vpool = ctx.enter_context(tc.tile_pool(name="vpool", bufs=6))
            pspool = ctx.enter_context(tc.tile_pool(name="psum", bufs=8, space="PSUM"))

            eps_t = singles.tile([P, 1], FP)
            nc.vector.memset(eps_t, EPS)

            def warmup(n_mm=16, width=512):
                """Emit garbage matmuls at t~1us so the sim's PE p-state
                ramp (low->mid->full over ~4.8us from the start of the
                current PE busy run) is burned while the PE would be idle
                waiting on weight DMA+prep.  Together with warm_gate()
                pings this keeps the PE "run" alive until the first real
                conv matmul, which then runs at full speed.  Inputs are a
                memset fp8 tile (products are 0); the PSUM scratch uses
                the shared "ps" ring and is never read."""
                wz = singles.tile([P, CH, width], F8, tag="wuw", name="wuw")
                nc.gpsimd.memset(wz, 0.0)
                ps = pspool.tile([P, width], FP, tag="ps", name="wups")
                for _ in range(n_mm):
                    nc.tensor.matmul(
                        ps, wz[:, :, 0:P], wz, start=True, stop=True,
                        perf_mode=DR, skip_group_check=True,
                    )

            def warm_gate(gate, fp32=True):
                """One tiny matmul whose rhs/lhsT read `gate` (a [128, >=128]
                fp32 AP, or fp8 with fp32=False): becomes READY when the
                gate tile is written, pinging the PE so its busy-run (and
                thus the p-state ramp credit) survives the head's idle
                stretches between DMA arrivals."""
                ps = pspool.tile([P, 16], FP, tag="ps", name="wgps")
                nc.tensor.matmul(
                    ps, gate[:, 0:P], gate[:, 0:16], start=True, stop=True,
                    skip_group_check=True,
                )

            # ---- weight prep (transposed host layout [ci, co, tap]):
            # binarize vs the per-co mean over (ci, tap).  Pipelined per
            # co-chunk so conv1's first matmuls can start before the whole
            # weight tensor is processed; see prep_math for the engine
            # split.  The conv matmul lhsT reads sgn[:, :, :, t] directly
            # (strided AP).
            wdr = {}
            s_t = {}
            t_t = {}

            wstg = {}

            def prep_dma_q(i, c, b):
                """One quarter (ci-chunk b of co-chunk c) of conv i's raw
                weights.  Quarter-granularity lets x transfers interleave
                into the weight stream on the serial DMA device."""
                if (i, c) not in wstg:
                    wstg[(i, c)] = wstage.tile(
                        [P, CH, P, 9], FP, tag=f"wtraw{c}", name=f"wtraw{i}_{c}"
                    )
                nc.sync.dma_start(
                    out=wstg[(i, c)][:, b], in_=w_ext[i][b][:, c * P : (c + 1) * P]
                )

            def prep_dma(i, c):
                for b in range(CH):
                    prep_dma_q(i, c, b)

            def prep_reduce(i, c):
                # tap-reduce (DVE) of the raw weight tile; emitted for both
                # co-chunks before any encode so the chunk-1 reduces are not
                # queued behind chunk-0's DVE encode ops.
                wT = wstg[(i, c)]
                tap = wstage.tile([P, CH, P], FP, tag=f"tap{c}")
                for b in range(CH):
                    nc.vector.tensor_reduce(
                        out=tap[:, b], in_=wT[:, b], axis=mybir.AxisListType.X,
                        op=mybir.AluOpType.add,
                    )
                wstg[(i, c, "tap")] = tap

            def prep_math(i, c):
                # mean via gpsimd partition all-reduce of the tap sums:
                # no PE matmuls, no PSUM — the PE queue stays pure conv work.
                # Compare w*2304 >= colsum instead of w >= mean (same ulp-level
                # rounding class as an explicit division).
                if (i, c, "tap") not in wstg:
                    prep_reduce(i, c)
                wT = wstg.pop((i, c))
                tap = wstg.pop((i, c, "tap"))
                asum = wstage.tile([P, CH, P], FP, tag=f"asum{c}")
                nc.gpsimd.partition_all_reduce(
                    asum, tap, channels=P, reduce_op=bass_isa.ReduceOp.add
                )
                csum = wstage.tile([P, P], FP, tag=f"csum{c}")
                nc.gpsimd.tensor_tensor(
                    out=csum, in0=asum[:, 0], in1=asum[:, 1], op=mybir.AluOpType.add
                )
                sgn = wpool.tile(
                    [P, CH, P, 9], F8, tag=f"sgn{i}_{c}", name=f"sgn{i}_{c}"
                )
                mean = wstage.tile([P, P], FP, tag=f"mean{c}")
                nc.gpsimd.tensor_scalar_mul(mean, csum, 1.0 / (CH * P * 9))
                if i == 1:
                    # conv1 (head-critical): bw in {-.5,+.5} (s1 is doubled),
                    # avoiding ACT which is busy with the x binarizations.
                    # b0 on DVE: fused (w*2304 >= colsum) then -0.5.
                    nc.vector.scalar_tensor_tensor(
                        out=sgn[:, 0], in0=wT[:, 0], scalar=float(CH * P * 9),
                        in1=csum.to_broadcast([P, P, 9]),
                        op0=mybir.AluOpType.mult, op1=mybir.AluOpType.is_ge,
                    )
                    nc.vector.tensor_scalar(
                        out=sgn[:, 0], in0=sgn[:, 0], scalar1=0.5, scalar2=None,
                        op0=mybir.AluOpType.subtract,
                    )
                    # b1: d = w - mean (sign-exact), fused (d >= 0) - 0.5;
                    # split by co-half across Pool and DVE — the Pool
                    # tensor_tensor runs at 0.42 efficiency and this chain
                    # is the head-critical path
                    d = wstage.tile([P, P, 9], FP, tag=f"d1_{c}")
                    hb = P // 2
                    for q, eng in ((0, nc.gpsimd), (1, nc.vector)):
                        sl = slice(q * hb, (q + 1) * hb)
                        eng.tensor_tensor(
                            out=d[:, sl], in0=wT[:, 1, sl],
                            in1=mean[:, sl].to_broadcast([P, hb, 9]),
                            op=mybir.AluOpType.subtract,
                        )
                        eng.tensor_scalar(
                            out=sgn[:, 1, sl], in0=d[:, sl], scalar1=0.0,
                            scalar2=0.5, op0=mybir.AluOpType.is_ge,
                            op1=mybir.AluOpType.subtract,
                        )
                else:
                    # conv2: d on DVE/Pool, then either sign to +-1 on ACT
                    # (w2_enc="act") or the {-.5,+.5} fused is_ge encode on
                    # DVE/Pool (w2_enc="ts", s2 doubled) to keep ACT free
                    # for the x binarizations + ba2 posts in the congested
                    # transition window.  b1's subtract is co-half split
                    # across Pool+DVE (Pool tensor_tensor runs at 0.42
                    # efficiency; this chain gates back(0))
                    hb = P // 2
                    for b in range(CH):
                        d = wstage.tile([P, P, 9], FP, tag=f"d{b}_{c}")
                        if b == 0:
                            nc.vector.tensor_tensor(
                                out=d, in0=wT[:, b],
                                in1=mean.to_broadcast([P, P, 9]),
                                op=mybir.AluOpType.subtract,
                            )
                        else:
                            for q, eng in ((0, nc.gpsimd), (1, nc.vector)):
                                sl = slice(q * hb, (q + 1) * hb)
                                eng.tensor_tensor(
                                    out=d[:, sl], in0=wT[:, b, sl],
                                    in1=mean[:, sl].to_broadcast([P, hb, 9]),
                                    op=mybir.AluOpType.subtract,
                                )
                        if w2_enc == "act" or (w2_enc == "ts1" and c == 0):
                            nc.scalar.sign(sgn[:, b], d)
                        else:
                            for q, eng in ((0, nc.gpsimd), (1, nc.vector)):
                                sl = slice(q * hb, (q + 1) * hb)
                                eng.tensor_scalar(
                                    out=sgn[:, b, sl], in0=d[:, sl],
                                    scalar1=0.0, scalar2=0.5,
                                    op0=mybir.AluOpType.is_ge,
                                    op1=mybir.AluOpType.subtract,
                                )
                for t in range(9):
                    wdr[(i, t, c)] = sgn[:, :, :, t]

            # ---- BN constants: s = alpha*gamma/sqrt(var+eps),
            #                    t = beta - mean*gamma/sqrt(var+eps)
            def prep_bn(i):
                bn_t = singles.tile([P, CH * 5], FP, tag=f"bn{i}", name=f"bn{i}")
                nc.sync.dma_start(out=bn_t, in_=bn_ext[i])
                for c in range(CH):
                    ld = {
                        nm: bn_t[:, c * 5 + k : c * 5 + k + 1]
                        for k, nm in enumerate(("alpha", "gamma", "beta", "mean", "var"))
                    }
                    std = singles.tile([P, 1], FP, tag=f"std{i}_{c}", name=f"std{i}_{c}")
                    nc.scalar.activation(std, ld["var"], AF.Sqrt, bias=eps_t)
                    g = singles.tile([P, 1], FP, tag=f"g{i}_{c}", name=f"g{i}_{c}")
                    nc.vector.reciprocal(g, std)
                    nc.vector.tensor_mul(g, g, ld["gamma"])
                    s = singles.tile([P, 1], FP, tag=f"s{i}_{c}", name=f"s{i}_{c}")
                    nc.vector.tensor_mul(s, g, ld["alpha"])
                    if i == 1:
                        nc.vector.tensor_add(s, s, s)  # x2: conv1 bw +-0.5
                        s4 = singles.tile(
                            [P, 1], FP, tag=f"s4_{c}", name=f"s4_{c}"
                        )
                        # x4 scale for images whose ba1 is half-encoded
                        nc.vector.tensor_add(s4, s, s)
                        s_t[("1x2", c)] = s4
                    elif w2_enc == "ts" or (w2_enc == "ts1" and c == 1):
                        nc.vector.tensor_add(s, s, s)  # x2: conv2 bw +-0.5
                    tt = singles.tile([P, 1], FP, tag=f"t{i}_{c}", name=f"t{i}_{c}")
                    nc.vector.tensor_mul(tt, g, ld["mean"])
                    nc.vector.tensor_sub(tt, ld["beta"], tt)
                    s_t[(i, c)] = s
                    t_t[(i, c)] = tt

            # ---- per-image stages -------------------------------------
            def xload(n):
                """x(n) DMA triggers on the SP queue.  Image 0's second
                chunk is split in two transfers so its binarization (the
                head's last gate) can start on the first half sooner."""
                xt = {}
                for b in range(CH):
                    xt[b] = xpool.tile([P, PIX], FP, tag=f"x{b}", name=f"x{b}")
                    nc.sync.dma_start(out=xt[b], in_=x_ext[n, b])
                return xt

            def basign(state, mode="act"):
                """ba1(n): +-1 sign on ACT ("act"), or the {-.5,+.5}
                encoding via one fused tensor_scalar per chunk — "half"
                puts both chunks on DVE (plain-ts runs at the 2x_2p rate
                there), "half-mixed" splits DVE/Pool.  conv1's post
                compensates half encodings with a doubled scale."""
                xt = state
                half = mode != "act"
                ba1 = bapool.tile([P, CH, BASTRIDE], F8, tag="ba1", name="ba1")
                for b in range(CH):
                    if half:
                        eng = (
                            nc.gpsimd
                            if (mode == "half-mixed" and b == 1)
                            else nc.vector
                        )
                        eng.tensor_scalar(
                            out=ba1[:, b, 0:PIX], in0=xt[b], scalar1=0.0,
                            scalar2=0.5, op0=mybir.AluOpType.is_ge,
                            op1=mybir.AluOpType.subtract,
                        )
                    elif xt.get(("split", b)):
                        hp = (HALF + 1) * W
                        for sl in (slice(0, hp), slice(hp, PIX)):
                            nc.scalar.sign(
                                ba1[:, b, sl], xt[b][:, sl],
                            )
                    else:
                        nc.scalar.sign(ba1[:, b, 0:PIX], xt[b])
                return xt, ba1, half

            def xpadd(xt, gate=None):
                """xp(n) = x(n) + t2 on GPSIMD (residual + BN shift).

                gate: an AP written late (e.g. a ba2 column).  The t2 scalar
                is routed through a tiny gate op that reads it, so the xp
                ops only become READY once conv1(n) is under way — keeping
                Pool free during the DMA-bound transition window (the
                scheduler dispatches by readiness, not emission order).
                """
                xp = {}
                for b in range(CH):
                    t2b = t_t[(2, b)]
                    if gate is not None:
                        t2l = xppool.tile([P, 1], FP, tag=f"t2l{b}", name=f"t2l{b}")
                        nc.vector.scalar_tensor_tensor(
                            out=t2l, in0=gate, scalar=0.0, in1=t2b,
                            op0=mybir.AluOpType.mult, op1=mybir.AluOpType.add,
                        )
                        t2b = t2l
                    xp[b] = xppool.tile([P, PIX], FP, tag=f"xp{b}", name=f"xp{b}")
                    nc.gpsimd.tensor_scalar_add(xp[b], xt[b], t2b)
                return xp

            def make_c1(n, state):
                """conv1(n) -> ba2(n); returns (shared-state, chunk-emitter)."""
                xt, ba1, half = state
                skey = "1x2" if half else 1
                st = {}

                def chunk(c):
                    first = not st
                    if first:
                        st["ba2"] = bapool.tile(
                            [P, CH, BASTRIDE], F8, tag="ba2", name="ba2"
                        )
                        st["xt"] = xt

                    def conv1_post(cc, h, hs, nr, ps):
                        # ba2 = sign(s1*conv + t1)  (sign(hardtanh(y))==sign(y))
                        nc.scalar.activation(
                            st["ba2"][:, cc, hs * W : hs * W + nr * W],
                            ps,
                            AF.Sign,
                            bias=t_t[(1, cc)],
                            scale=s_t[(skey, cc)],
                        )

                    _conv_chunk(nc, pspool, wdr, 1, ba1, c, conv1_post)
                    if c == 1:
                        # emitted after the second chunk so the gate read
                        # depends on the last ba2 write (see xpadd)
                        st["xp"] = xpadd(
                            xt, gate=st["ba2"][:, 1, PIX - 1 : PIX]
                        )

                return st, chunk

            def make_back(n, st, last=False):
                def chunk(c):
                    if "xp" not in st:
                        st["xp"] = xpadd(st["xt"])
                    def conv2_post(cc, h, hs, nr, ps):
                        # last image's epilogue is the tail: put the clamp of
                        # alternating halves on Pool (GPSIMD cannot read
                        # PSUM, so the stt stays on DVE)
                        eng = nc.gpsimd if (last and (cc + h) % 2) else nc.vector
                        v = vpool.tile([P, nr * W], BF, tag="v", name="v")
                        nc.vector.scalar_tensor_tensor(
                            out=v, in0=ps, scalar=s_t[(2, cc)],
                            in1=st["xp"][cc][:, hs * W : hs * W + nr * W],
                            op0=mybir.AluOpType.mult, op1=mybir.AluOpType.add,
                        )
                        eng.tensor_scalar(
                            out=v, in0=v, scalar1=1.0, scalar2=-1.0,
                            op0=mybir.AluOpType.min, op1=mybir.AluOpType.max,
                        )
                        nc.sync.dma_start(
                            out=out_ext[n, cc][:, hs * W : hs * W + nr * W],
                            in_=v,
                        )

                    if last and (c == 1 or tail_rb0) and tail_rb:
                        # pipeline the final chunk's epilogue with its conv
                        # (4-row blocks) but keep one store per 16-row half:
                        # 8 small stores would serialize on the 625ns HWDGE
                        vh = {}

                        def conv2_post_last(cc, h, hs, nr, ps):
                            q = hs // HALF  # which 16-row half
                            if q not in vh:
                                vh[q] = vpool.tile(
                                    [P, HALF * W], BF, tag="v", name="vlast"
                                )
                            ro = (hs % HALF) * W
                            sl = slice(ro, ro + nr * W)
                            v = vh[q]
                            nc.vector.scalar_tensor_tensor(
                                out=v[:, sl], in0=ps, scalar=s_t[(2, cc)],
                                in1=st["xp"][cc][:, hs * W : hs * W + nr * W],
                                op0=mybir.AluOpType.mult,
                                op1=mybir.AluOpType.add,
                            )
                            eng = nc.gpsimd if h % 2 else nc.vector
                            eng.tensor_scalar(
                                out=v[:, sl], in0=v[:, sl], scalar1=1.0,
                                scalar2=-1.0, op0=mybir.AluOpType.min,
                                op1=mybir.AluOpType.max,
                            )
                            if hs + nr == (q + 1) * HALF:
                                nc.sync.dma_start(
                                    out=out_ext[n, cc][
                                        :, q * HALF * W : (q + 1) * HALF * W
                                    ],
                                    in_=v,
                                )

                        _conv_chunk(nc, pspool, wdr, 2, st["ba2"], c,
                                    conv2_post_last, rblock=tail_rb)
                    else:
                        _conv_chunk(nc, pspool, wdr, 2, st["ba2"], c,
                                    conv2_post)

                return chunk

            def everything(_iv=None):
                # Emission strategy: the PE executes in strict emission
                # order (PSUM accumulation groups), while the other engines
                # are re-scheduled by readiness (with a small wait-queue
                # bypass).  The program below is one token list: "dwq"/"dx"
                # tokens define the serial-DMA device order, "c1"/"back"
                # tokens the PE order, "sgn"/"pm" the vector-engine queue
                # positions.  x transfers interleave into the weight DMA
                # stream so neither conv1 (x-gated) nor conv2 (w2-prep-
                # gated) starves the PE in the transition window.
                xts = {}
                c1s, backs, sts = {}, {}, {}

                def create(n):
                    if n not in c1s:
                        mode = half_mode if n in half_imgs else "act"
                        st, fn = make_c1(n, basign(xts.pop(n), mode))
                        sts[n], c1s[n] = st, fn

                def emit(kind, n, c):
                    # lazy creation keeps pool-buffer reuse correct: a
                    # buffer's next writer must be emitted after its
                    # previous readers
                    if kind == "c1":
                        create(n)
                        c1s[n](c)
                    else:
                        if n not in backs:
                            backs[n] = make_back(
                                n, sts[n], last=(n >= IMGS - 2)
                            )
                        backs[n](c)

                warmup()

                if not pipelined:
                    prep_bn(1)
                    prep_bn(2)
                    prep_dma(1, 0)
                    prep_dma(1, 1)
                    prep_math(1, 0)
                    prep_math(1, 1)
                    prep_dma(2, 0)
                    prep_dma(2, 1)
                    prep_math(2, 0)
                    prep_math(2, 1)
                    for n in range(IMGS):
                        xts[n] = xload(n)
                    for n in range(IMGS):
                        for kind in ("c1", "back"):
                            emit(kind, n, 0)
                            emit(kind, n, 1)
                    return

                if prog_variant == 0:
                    # replicates the tuned baseline order (+warmup/gates)
                    prog = [
                        ("dwq", 1, 0, 0), ("dwq", 1, 0, 1),
                        ("dwq", 1, 1, 0), ("dwq", 1, 1, 1),
                        ("wgw", 1, 0, 0), ("wgw", 1, 0, 1),
                        ("wgw", 1, 1, 0), ("wgw", 1, 1, 1),
                        ("bn", 1, 0), ("bn", 2, 0),
                        ("pm", 1, 0), ("wgs", 1, 0), ("pm", 1, 1),
                        ("dx", 0, 0), ("dx", 0, 1, "S"), ("wgx", 0),
                        ("dx", 1, 0),
                        (("dx", 1, 1, "s") if split_x1 else ("dx", 1, 1)),
                    ]
                    if reorder_x2:
                        prog += [
                            ("dx", 2, 0),
                            ("dwq", 2, 0, 0), ("dwq", 2, 0, 1),
                            ("dx", 2, 1),
                        ]
                    else:
                        prog += [
                            ("dwq", 2, 0, 0), ("dwq", 2, 0, 1),
                            ("dx", 2, 0), ("dx", 2, 1, "s"),
                        ]
                    prog += [
                        ("dwq", 2, 1, 0), ("dwq", 2, 1, 1),
                        ("dx", 3, 0), ("dx", 3, 1, "s"),
                        ("c1", 0, 0), ("c1", 1, 0), ("c1", 0, 1),
                        ("sgn", 2),
                        ("c1", 1, 1), ("pm", 2, 0),
                        ("c1", 2, 0),
                        ("sgn", 3),
                        ("c1", 2, 1), ("pm", 2, 1),
                    ]
                    # steady state: backs lead by one chunk so ready
                    # conv2 work covers every ba1(n)/w2-encode wait; c1(3,1)
                    # covers the w2c1-encode latency for back(0,1)
                    prog += [
                        ("dx", 4, 0), ("dx", 4, 1),
                        ("back", 0, 0), ("back", 1, 0),
                        ("c1", 3, 0), ("c1", 3, 1), ("back", 0, 1),
                        ("dx", 5, 0), ("dx", 5, 1),
                        ("back", 1, 1), ("c1", 4, 0), ("back", 2, 0), ("c1", 4, 1),
                        ("dx", 6, 0), ("dx", 6, 1),
                        ("back", 2, 1), ("c1", 5, 0), ("back", 3, 0), ("c1", 5, 1),
                        ("dx", 7, 0), ("dx", 7, 1),
                        ("back", 3, 1), ("c1", 6, 0), ("back", 4, 0), ("c1", 6, 1),
                        ("back", 4, 1), ("c1", 7, 0), ("back", 5, 0), ("c1", 7, 1),
                        ("back", 5, 1), ("back", 6, 0), ("back", 7, 0),
                        ("back", 6, 1), ("back", 7, 1),
                    ]
                else:
                    prog = [
                        ("dwq", 1, 0, 0), ("dwq", 1, 0, 1),
                        ("dwq", 1, 1, 0), ("dwq", 1, 1, 1),
                        ("wgw", 1, 0, 0), ("wgw", 1, 0, 1),
                        ("wgw", 1, 1, 0), ("wgw", 1, 1, 1),
                        ("bn", 1, 0), ("bn", 2, 0),
                        ("pm", 1, 0), ("wgs", 1, 0), ("pm", 1, 1),
                        ("dx", 0, 0), ("dx", 0, 1), ("wgx", 0), ("sgn", 0),
                        ("dx", 1, 0), ("dwq", 2, 0, 0),
                        ("dx", 1, 1), ("dwq", 2, 0, 1),
                        ("c1", 0, 0), ("c1", 0, 1),
                        ("sgn", 1),
                        ("dx", 2, 0), ("dx", 2, 1),
                        ("c1", 1, 0), ("c1", 1, 1),
                        ("pm", 2, 0),
                        ("sgn", 2),
                        ("dwq", 2, 1, 0), ("dx", 3, 0),
                        ("dwq", 2, 1, 1), ("dx", 3, 1),
                        ("c1", 2, 0), ("c1", 2, 1),
                        ("pm", 2, 1),
                        ("sgn", 3),
                        ("dx", 4, 0), ("dx", 4, 1),
                    ]
                    for n in range(IMGS - 3):
                        prog += [
                            ("back", n, 0), ("c1", n + 3, 0),
                            ("back", n, 1), ("c1", n + 3, 1),
                        ]
                        if n + 5 < IMGS:
                            prog += [("dx", n + 5, 0), ("dx", n + 5, 1)]
                        if n + 4 < IMGS:
                            prog += [("sgn", n + 4)]
                    for n in range(IMGS - 3, IMGS):
                        prog += [("back", n, 0), ("back", n, 1)]

                for tok in prog:
                    kind = tok[0]
                    if kind == "dwq":
                        prep_dma_q(tok[1], tok[2], tok[3])
                    elif kind == "dx":
                        n, b = tok[1], tok[2]
                        if n not in xts:
                            xts[n] = {}
                        xts[n][b] = xpool.tile(
                            [P, PIX], FP, tag=f"x{b}", name=f"x{b}"
                        )
                        if len(tok) > 3 and tok[3] == "S":
                            # sign-only split: one transfer, two sign ops
                            # (the transfer split costs an extra HWDGE slot
                            # + sem; the sign split alone is what unblocks
                            # the first conv half)
                            nc.sync.dma_start(out=xts[n][b], in_=x_ext[n, b])
                            xts[n]["split", b] = True
                        elif len(tok) > 3:
                            # split into two half transfers so the sign of
                            # the first half starts one transfer earlier
                            # split at row 17 (not 16): conv h0's dh=+1
                            # taps read ba row 16, so a 512-split makes h0
                            # cross into the second half-sign
                            hp = (HALF + 1) * W
                            for sl in (slice(0, hp), slice(hp, PIX)):
                                nc.sync.dma_start(
                                    out=xts[n][b][:, sl],
                                    in_=x_ext[n, b][:, sl],
                                )
                            xts[n]["split", b] = True
                        else:
                            nc.sync.dma_start(out=xts[n][b], in_=x_ext[n, b])
                    elif kind == "wgw":
                        warm_gate(
                            wstg[(tok[1], tok[2])][:, tok[3]].rearrange(
                                "p a b -> p (a b)"
                            )
                        )
                    elif kind == "wgs":
                        warm_gate(wdr[(tok[1], 0, tok[2])][:, 0])
                    elif kind == "wgx":
                        warm_gate(xts[tok[1]][0])
                    elif kind == "bn":
                        prep_bn(tok[1])
                    elif kind == "sgn":
                        create(tok[1])
                    elif kind == "pm":
                        prep_math(tok[1], tok[2])
                    else:
                        emit(kind, tok[1], tok[2])

            if loop_r is None:
                everything()
            else:
                with tc.For_i(0, loop_r, 1) as iv:
                    everything(iv)

    nc.compile()
    return nc


_NC_CACHE = None


def _get_program():
    global _NC_CACHE
    if _NC_CACHE is None:
        _NC_CACHE = build_program()
    return _NC_CACHE


def make_in_maps(inputs):
    x = np.ascontiguousarray(inputs["x"], dtype=np.float32).reshape(
        N_CORES, IMGS, CH, P, PIX
    )
    shared = {}
    for i in (1, 2):
        # [co, ci, kh, kw] -> [ci, co, tap] -> chunked [CH, P, 256, 9]
        shared[f"conv{i}_w"] = np.ascontiguousarray(
            np.asarray(inputs[f"conv{i}_w"], dtype=np.float32)
            .reshape(CH * P, CH * P, 9)
            .transpose(1, 0, 2)
        ).reshape(CH, P, CH * P, 9)
        packed = np.stack(
            [
                np.asarray(inputs[f"alpha{i}"], dtype=np.float32).reshape(CH * P),
                np.asarray(inputs[f"bn{i}_gamma"], dtype=np.float32),
                np.asarray(inputs[f"bn{i}_beta"], dtype=np.float32),
                np.asarray(inputs[f"bn{i}_mean"], dtype=np.float32),
                np.asarray(inputs[f"bn{i}_var"], dtype=np.float32),
            ],
            axis=-1,
        ).reshape(CH, P, 5)
        # -> [co%128, co_chunk*5 + k]
        shared[f"bn{i}_all"] = np.ascontiguousarray(
            packed.transpose(1, 0, 2)
        ).reshape(P, CH * 5)
    return [{"x": x[c], **shared} for c in range(N_CORES)]


def kernel(**inputs):
    nc = _get_program()
    in_maps = make_in_maps(inputs)
    res = run_bass_kernel_spmd(nc, in_maps, list(range(N_CORES)))
    out = np.stack(
        [np.asarray(res.results[c]["out"]).astype(np.float32) for c in range(N_CORES)]
    )
    return out.reshape(N_CORES * IMGS, CH * P, H, W)

